# revision 2
# baseline (speedup 1.0000x reference)
# Trainium2 Bass kernel for nn_Attention_48052094107920 (sparse_attention).
#
# Math (see reference):
#   q,k: GH=3 global heads of dim 64; v: LH=12 local heads of dim 64
#   S_g = (x Wq)_g (x Wk)_g^T * scale                  [B,3,N,N]
#   mw  = (masks @ mask_proj).reshape(N,N,3,12)
#   A_h = sum_g S_g * mw[:,:,g,h]                      [B,12,N,N]
#   out = softmax_k(A_h) @ v_h  -> output projection + bias
#
# Default sharding (kernel_v2): core c = (head-group c//2, query-half c%2).
# Each core processes ALL 8 batches for its 3 heads and 320-column q-half and
# emits partial projection outputs; the host sums the 4 head-group partials
# and adds proj_b. This amortizes the batch-independent mask-mix maps
# mw[g,h] (the dominant Vector-engine work) over all 8 batches.
# kernel_v1 (kept below) is plain batch-parallel: 1 batch per core.
#
# Shared device-side design choices:
#   - "k-major" score layout: score tiles are S^T[k, q] (k on partitions) so
#     p @ v needs no transposes and the output projection receives its lhsT
#     (= o^T) directly from PSUM.
#   - x is host-transposed and augmented with a ones-row; W_v is augmented so
#     the v projection yields interleaved [v_h | ones] columns: the ones
#     column produces the softmax denominator Z during the p@v matmul.
#   - softmax skips max-subtraction (logits are O(5) here) and folds 1/Z in
#     after p@v; padded k-rows are killed with a -30 exp bias.
#   - fp16 everywhere on-chip (fp32 PSUM accumulation and fp32 Z / 1/Z);
#     mask_proj enters as per-partition scalar tiles, so no runtime values
#     are baked into the compiled program.

import numpy as np
import ml_dtypes

import concourse.bass as bass
import concourse.bacc as bacc_mod
import concourse.mybir as mybir
import concourse.tile as tile
from concourse import bass_utils

BF = mybir.dt.float16  # fp16: 10-bit mantissa, same engine speed as bf16
F32 = mybir.dt.float32
AF = mybir.ActivationFunctionType
OP = mybir.AluOpType

B, N, C = 8, 577, 768
GH, LH, ML, HD = 3, 12, 3, 64
SCALE = HD ** -0.5
NP = 640          # padded token count (5 * 128)
NJ = 5            # k sub-chunks of 128
CP = 896          # padded channel count (7 * 128)
KO = 7            # contraction sub-chunks for the qkv projections
QC = 128          # q-chunk width
NQC = NP // QC    # 5 q-chunks
VW = HD + 1       # 65: v head columns + ones column
EXP_NEG = -30.0   # exp bias for padded k rows


def build_nc(linearize=False):
    nc = bacc_mod.Bacc("TRN2", target_bir_lowering=False, debug=False, num_devices=8)

    xt = nc.dram_tensor("xt", [128, KO, NP], BF, kind="ExternalInput")
    wq = nc.dram_tensor("wq", [128, KO, GH * HD], BF, kind="ExternalInput")
    wk = nc.dram_tensor("wk", [128, KO, GH * HD], BF, kind="ExternalInput")
    wv = nc.dram_tensor("wv", [128, KO, LH * VW], BF, kind="ExternalInput")
    pw = nc.dram_tensor("pw", [64, LH, C], BF, kind="ExternalInput")
    pb = nc.dram_tensor("pb", [128, C], F32, kind="ExternalInput")
    mt = nc.dram_tensor("mt", [128, ML * NJ, NP], BF, kind="ExternalInput")
    cw = nc.dram_tensor("cw", [128, GH * ML * LH], F32, kind="ExternalInput")
    eb = nc.dram_tensor("eb", [128, 1], F32, kind="ExternalInput")
    out = nc.dram_tensor("o", [NP, C], F32, kind="ExternalOutput")

    with tile.TileContext(nc, linearize=linearize) as tc, \
         tc.tile_pool(name="const", bufs=1) as cpool, \
         tc.tile_pool(name="work", bufs=2) as wpool, \
         tc.tile_pool(name="attn", bufs=3) as apool, \
         tc.tile_pool(name="psA", bufs=2, space="PSUM") as ppA, \
         tc.tile_pool(name="psS", bufs=4, space="PSUM") as ppS, \
         tc.tile_pool(name="psO", bufs=2, space="PSUM") as ppO:

        # ---- load constants ----
        xt_s = cpool.tile([128, KO, NP], BF, tag="xt")
        nc.sync.dma_start(xt_s[:], xt.ap())
        wq_s = cpool.tile([128, KO, GH * HD], BF, tag="wq")
        nc.sync.dma_start(wq_s[:], wq.ap())
        wk_s = cpool.tile([128, KO, GH * HD], BF, tag="wk")
        nc.sync.dma_start(wk_s[:], wk.ap())
        wv_s = cpool.tile([128, KO, LH * VW], BF, tag="wv")
        nc.sync.dma_start(wv_s[:], wv.ap())
        pw_s = cpool.tile([64, LH, C], BF, tag="pw")
        nc.sync.dma_start(pw_s[:], pw.ap())
        pb_s = cpool.tile([128, C], F32, tag="pb")
        nc.sync.dma_start(pb_s[:], pb.ap())
        mt_s = cpool.tile([128, ML * NJ, NP], BF, tag="mt")
        nc.sync.dma_start(mt_s[:], mt.ap())
        cw_s = cpool.tile([128, GH * ML * LH], F32, tag="cw")
        nc.sync.dma_start(cw_s[:], cw.ap())
        eb_s = cpool.tile([128, 1], F32, tag="eb")
        nc.sync.dma_start(eb_s[:], eb.ap())

        # ---- phase A: projections ----
        # qT/kT: [d, token] per head; heads g0,g1 packed on partitions 0:64 /
        # 64:128 of one [128, NP] tile, g2 on its own [64, NP] tile.
        qT01 = cpool.tile([128, NP], BF, tag="qT01")
        qT2 = cpool.tile([64, NP], BF, tag="qT2")
        kT01 = cpool.tile([128, NP], BF, tag="kT01")
        kT2 = cpool.tile([64, NP], BF, tag="kT2")
        vt = cpool.tile([128, NJ, LH * VW], BF, tag="vt")

        def proj_to(dst, w_s, mslc, mpart):
            # dst[token-cols] = (W[:, mslc]^T @ xhatT), written as fp16
            for n0, n1 in ((0, 512), (512, NP)):
                ps = ppA.tile([128, 512], F32, tag="bigA", name="psA")[:mpart, : n1 - n0]
                for o in range(KO):
                    nc.tensor.matmul(
                        ps, w_s[:, o, mslc], xt_s[:, o, n0:n1],
                        start=(o == 0), stop=(o == KO - 1),
                    )
                nc.scalar.copy(dst[:mpart, n0:n1], ps)

        proj_to(qT01, wq_s, slice(0, 128), 128)
        proj_to(qT2, wq_s, slice(128, 192), 64)
        proj_to(kT01, wk_s, slice(0, 128), 128)
        proj_to(kT2, wk_s, slice(128, 192), 64)

        # v-hat: rows k (5 chunks of 128), cols = 12 heads x [v | ones]
        for kc in range(NJ):
            for n0, n1 in ((0, 512), (512, LH * VW)):
                ps = ppA.tile([128, 512], F32, tag="bigA", name="psA")[:, : n1 - n0]
                for o in range(KO):
                    nc.tensor.matmul(
                        ps, xt_s[:, o, kc * 128:(kc + 1) * 128], wv_s[:, o, n0:n1],
                        start=(o == 0), stop=(o == KO - 1),
                    )
                nc.scalar.copy(vt[:, kc, n0:n1], ps)

        def qg(g):
            return (qT01[0:64], qT01[64:128], qT2[0:64])[g]

        def kg(g):
            return (kT01[0:64], kT01[64:128], kT2[0:64])[g]

        # ---- phase B: per q-chunk pipeline ----
        for qc in range(NQC):
            qs = slice(qc * QC, (qc + 1) * QC)

            # scores S^T[k, q] for each global head -> bf16 SBUF
            ssb = wpool.tile([128, GH, NJ, QC], BF, tag="ssb")
            for g in range(GH):
                for j in range(NJ):
                    ps = ppS.tile([128, QC], F32, tag="s", name="psS")
                    nc.tensor.matmul(
                        ps, kg(g)[:, j * 128:(j + 1) * 128], qg(g)[:, qs],
                        start=True, stop=True,
                    )
                    nc.scalar.copy(ssb[:, g, j], ps)

            # channel maps c[g,m] = S_g * masks_m
            cmaps = wpool.tile([128, GH * ML, NJ, QC], BF, tag="cmaps")
            for g in range(GH):
                for m in range(ML):
                    nc.vector.tensor_mul(
                        cmaps[:, g * ML + m], ssb[:, g],
                        mt_s[:, m * NJ:(m + 1) * NJ, qs],
                    )

            # per-head mix + exp + p@v
            osb = wpool.tile([64, LH, QC], F32, tag="osb")
            zsb = wpool.tile([65, LH, QC], F32, tag="zsb")
            zrec = wpool.tile([65, LH, QC], F32, tag="zrec")
            for h in range(LH):
                at = apool.tile([128, NJ, QC], BF, tag="at")
                # attn_h = sum_{g,m} mask_proj[m, g*LH+h] * c[g,m]
                i0 = 0  # channel (g=0, m=0)
                nc.vector.tensor_scalar(
                    at[:], cmaps[:, i0], cw_s[:, h:h + 1], None, OP.mult,
                )
                for g in range(GH):
                    for m in range(ML):
                        if g == 0 and m == 0:
                            continue
                        idx = (g * ML + m) * LH + h
                        nc.vector.scalar_tensor_tensor(
                            out=at[:], in0=cmaps[:, g * ML + m],
                            scalar=cw_s[:, idx:idx + 1], in1=at[:],
                            op0=OP.mult, op1=OP.add,
                        )
                e = apool.tile([128, NJ, QC], BF, tag="e")
                nc.scalar.activation(e[:, 0:4], at[:, 0:4], AF.Exp)
                nc.scalar.activation(e[:, 4:5], at[:, 4:5], AF.Exp, bias=eb_s[:, 0:1])

                # o^T_h (and Z in row 64) = vhat_h^T @ e
                pov = ppO.tile([VW, QC], F32, tag="ov")
                for j in range(NJ):
                    nc.tensor.matmul(
                        pov, vt[:, j, h * VW:(h + 1) * VW], e[:, j, :],
                        start=(j == 0), stop=(j == NJ - 1),
                    )
                nc.scalar.copy(osb[:, h], pov[0:64])
                nc.scalar.copy(zsb[64:65, h], pov[64:65])

            # 1/Z, broadcast over the 64 head-dim partitions via DMA
            nc.vector.reciprocal(zrec[64:65], zsb[64:65])
            zrep = wpool.tile([64, LH, QC], F32, tag="zrep")
            nc.sync.dma_start(
                zrep[:], zrec[64:65, None, :, :].to_broadcast((1, 64, LH, QC))
            )
            on = wpool.tile([64, LH, QC], BF, tag="on")
            nc.vector.tensor_mul(on[:], osb[:], zrep[:])

            # final projection for this q-chunk (+ bias)
            outsb = wpool.tile([128, C], F32, tag="outsb")
            for n0, n1 in ((0, 512), (512, C)):
                ps = ppA.tile([128, 512], F32, tag="bigA", name="psA")[:, : n1 - n0]
                for kk in range(LH):
                    nc.tensor.matmul(
                        ps, on[:, kk, :], pw_s[:, kk, n0:n1],
                        start=(kk == 0), stop=(kk == LH - 1),
                    )
                nc.vector.tensor_add(outsb[:, n0:n1], ps, pb_s[:, n0:n1])
            nc.sync.dma_start(
                out.ap().rearrange("(j p) c -> p j c", p=128)[:, qc, :], outsb[:]
            )

    nc.compile()
    return nc


def prep_xt(x_b):
    bf = np.float16
    xhatT = np.zeros((CP, NP), np.float32)
    xhatT[:C, :N] = x_b.T
    xhatT[C, :N] = 1.0
    xt = np.ascontiguousarray(xhatT.reshape(KO, 128, NP).transpose(1, 0, 2))
    return xt.astype(bf)


def prep_shared_inputs(masks, Wq, Wk, Wv, mask_proj, proj_w, proj_b):
    """Host-side prep of the batch-independent input tensors."""
    bf = np.float16

    def wpad(w, scale=1.0):
        wp = np.zeros((CP, w.shape[1]), np.float32)
        wp[:C] = w * scale
        return np.ascontiguousarray(wp.reshape(KO, 128, -1).transpose(1, 0, 2))

    wqp = wpad(Wq, SCALE)
    wkp = wpad(Wk)

    wvh = np.zeros((CP, LH * VW), np.float32)
    for h in range(LH):
        wvh[:C, h * VW:h * VW + HD] = Wv[:, h * HD:(h + 1) * HD]
        wvh[C, h * VW + HD] = 1.0
    wvp = np.ascontiguousarray(wvh.reshape(KO, 128, -1).transpose(1, 0, 2))

    pwp = np.ascontiguousarray(proj_w.reshape(LH, 64, C).transpose(1, 0, 2))
    pbp = np.broadcast_to(proj_b.astype(np.float32), (128, C)).copy()

    # mt[p, m*NJ+j, t] = masks[t, j*128+p, m]  (zero padded)
    mtp = np.zeros((128, ML * NJ, NP), np.float32)
    mk = masks.transpose(2, 1, 0)  # [m, k, q]
    mkp = np.zeros((ML, NP, NP), np.float32)
    mkp[:, :N, :N] = mk
    mtp[:] = mkp.reshape(ML, NJ, 128, NP).transpose(2, 0, 1, 3).reshape(128, ML * NJ, NP)

    # cw[p, (g*ML+m)*LH + h] = mask_proj[m, g*LH + h]
    cwv = np.zeros(GH * ML * LH, np.float32)
    for g in range(GH):
        for m in range(ML):
            for h in range(LH):
                cwv[(g * ML + m) * LH + h] = mask_proj[m, g * LH + h]
    cwp = np.broadcast_to(cwv, (128, GH * ML * LH)).copy()

    ebp = np.zeros((128, 1), np.float32)
    ebp[65:, 0] = EXP_NEG  # k = 512 + p valid through p = 64 (k = 576)

    return {
        "wq": wqp.astype(bf), "wk": wkp.astype(bf),
        "wv": wvp.astype(bf), "pw": pwp.astype(bf), "pb": pbp,
        "mt": mtp.astype(bf), "cw": cwp, "eb": ebp,
    }


_NC = None
_LINEARIZE = False


def get_nc():
    global _NC
    if _NC is None:
        _NC = build_nc(linearize=_LINEARIZE)
    return _NC


def kernel_v1(x, masks, Wq, Wk, Wv, mask_proj, proj_w, proj_b):
    x = np.asarray(x, np.float32)
    masks = np.asarray(masks, np.float32)
    Wq = np.asarray(Wq, np.float32)
    Wk = np.asarray(Wk, np.float32)
    Wv = np.asarray(Wv, np.float32)
    mask_proj = np.asarray(mask_proj, np.float32)
    proj_w = np.asarray(proj_w, np.float32)
    proj_b = np.asarray(proj_b, np.float32)

    shared = prep_shared_inputs(masks, Wq, Wk, Wv, mask_proj, proj_w, proj_b)
    in_maps = [dict(shared, xt=prep_xt(x[b])) for b in range(B)]

    res = bass_utils.run_bass_kernel_spmd(get_nc(), in_maps, core_ids=list(range(B)))
    out = np.stack([r["o"][:N, :] for r in res.results])
    return out.astype(np.float32)




# ======================================================================
# Stage-2 sharding (default): 4 head-groups x 2 query-halves.
# ======================================================================

B, N, C = 8, 577, 768
GH, LH, ML, HD = 3, 12, 3, 64
NH = 3            # heads per core
SCALE = HD ** -0.5
NP, NJ = 640, 5
CP, KO = 896, 7
QW = 320          # q-half width
VW = HD + 1
EXP_NEG = -30.0
QCHUNKS = ((0, 128), (128, 256), (256, 320))  # local q chunks for proj/psum


def build_nc2():
    nc = bacc_mod.Bacc("TRN2", target_bir_lowering=False, debug=False, num_devices=8)

    xta = nc.dram_tensor("xta", [B, 128, KO, NP], BF, kind="ExternalInput")
    xqa = nc.dram_tensor("xqa", [B, 128, KO, QW], BF, kind="ExternalInput")
    wq = nc.dram_tensor("wq", [128, KO, GH * HD], BF, kind="ExternalInput")
    wk = nc.dram_tensor("wk", [128, KO, GH * HD], BF, kind="ExternalInput")
    wv = nc.dram_tensor("wv", [128, KO, NH * VW], BF, kind="ExternalInput")
    pw = nc.dram_tensor("pw", [64, NH, C], BF, kind="ExternalInput")
    mt = nc.dram_tensor("mt", [128, ML * NJ, QW], BF, kind="ExternalInput")
    cw = nc.dram_tensor("cw", [128, GH * ML * NH], F32, kind="ExternalInput")
    eb = nc.dram_tensor("eb", [128, 1], F32, kind="ExternalInput")
    out = nc.dram_tensor("op", [B, QW, C], BF, kind="ExternalOutput")

    with tile.TileContext(nc) as tc, \
         tc.tile_pool(name="const", bufs=1) as cpool, \
         tc.tile_pool(name="xb", bufs=2) as xpool, \
         tc.tile_pool(name="work", bufs=2) as wpool, \
         tc.tile_pool(name="attn", bufs=3) as apool, \
         tc.tile_pool(name="psA", bufs=2, space="PSUM") as ppA, \
         tc.tile_pool(name="psS", bufs=1, space="PSUM") as ppS, \
         tc.tile_pool(name="psO", bufs=1, space="PSUM") as ppO, \
         tc.tile_pool(name="psP", bufs=1, space="PSUM") as ppP:

        wq_s = cpool.tile([128, KO, GH * HD], BF, tag="wq")
        nc.sync.dma_start(wq_s[:], wq.ap())
        wk_s = cpool.tile([128, KO, GH * HD], BF, tag="wk")
        nc.sync.dma_start(wk_s[:], wk.ap())
        wv_s = cpool.tile([128, KO, NH * VW], BF, tag="wv")
        nc.sync.dma_start(wv_s[:], wv.ap())
        pw_s = cpool.tile([64, NH, C], BF, tag="pw")
        nc.sync.dma_start(pw_s[:], pw.ap())
        mt_s = cpool.tile([128, ML * NJ, QW], BF, tag="mt")
        nc.sync.dma_start(mt_s[:], mt.ap())
        cw_s = cpool.tile([128, GH * ML * NH], F32, tag="cw")
        nc.sync.dma_start(cw_s[:], cw.ap())
        eb_s = cpool.tile([128, 1], F32, tag="eb")
        nc.sync.dma_start(eb_s[:], eb.ap())

        # mw[g,hh] = sum_m mask_proj[m, g*LH + H0+hh] * masks_m  (batch-free)
        mwsb = cpool.tile([128, GH * NH, NJ, QW], BF, tag="mw")
        for g in range(GH):
            for hh in range(NH):
                d = mwsb[:, g * NH + hh]
                i0 = (g * ML + 0) * NH + hh
                nc.vector.tensor_scalar(
                    d, mt_s[:, 0:NJ], cw_s[:, i0:i0 + 1], None, OP.mult,
                )
                for m in (1, 2):
                    im = (g * ML + m) * NH + hh
                    nc.vector.scalar_tensor_tensor(
                        out=d, in0=mt_s[:, m * NJ:(m + 1) * NJ],
                        scalar=cw_s[:, im:im + 1], in1=d,
                        op0=OP.mult, op1=OP.add,
                    )

        def phase_a(b):
            xb = xpool.tile([128, KO, NP], BF, tag="xb")
            nc.sync.dma_start(xb[:], xta.ap()[b])
            xq = xpool.tile([128, KO, QW], BF, tag="xq")
            nc.sync.dma_start(xq[:], xqa.ap()[b])

            q01 = wpool.tile([128, QW], BF, tag="q01")
            q2 = wpool.tile([64, QW], BF, tag="q2")
            k01 = wpool.tile([128, NP], BF, tag="k01")
            k2 = wpool.tile([64, NP], BF, tag="k2")
            vtb = wpool.tile([128, NJ, NH * VW], BF, tag="vtb")

            for msl, mp, dst in ((slice(0, 128), 128, q01), (slice(128, 192), 64, q2)):
                ps = ppA.tile([128, 512], F32, tag="bigA", name="psA")[:mp, :QW]
                for o in range(KO):
                    nc.tensor.matmul(ps, wq_s[:, o, msl], xq[:, o, :],
                                     start=(o == 0), stop=(o == KO - 1))
                nc.scalar.copy(dst[:mp, :], ps)

            for msl, mp, dst in ((slice(0, 128), 128, k01), (slice(128, 192), 64, k2)):
                for n0, n1 in ((0, 512), (512, NP)):
                    ps = ppA.tile([128, 512], F32, tag="bigA", name="psA")[:mp, : n1 - n0]
                    for o in range(KO):
                        nc.tensor.matmul(ps, wk_s[:, o, msl], xb[:, o, n0:n1],
                                         start=(o == 0), stop=(o == KO - 1))
                    nc.scalar.copy(dst[:mp, n0:n1], ps)

            for kc in range(NJ):
                ps = ppA.tile([128, 512], F32, tag="bigA", name="psA")[:, : NH * VW]
                for o in range(KO):
                    nc.tensor.matmul(ps, xb[:, o, kc * 128:(kc + 1) * 128], wv_s[:, o, :],
                                     start=(o == 0), stop=(o == KO - 1))
                nc.scalar.copy(vtb[:, kc, :], ps)
            return q01, q2, k01, k2, vtb

        def phase_b(b, q01, q2, k01, k2, vtb):
            def qg(g):
                return (q01[0:64], q01[64:128], q2[0:64])[g]

            def kg(g):
                return (k01[0:64], k01[64:128], k2[0:64])[g]

            ssb = wpool.tile([128, GH, NJ, QW], BF, tag="ssb")
            for g in range(GH):
                psa = ppS.tile([128, NJ, 256], F32, tag="s256", name="psS1")
                psb = ppS.tile([128, NJ, 64], F32, tag="s64", name="psS2")
                for j in range(NJ):
                    nc.tensor.matmul(psa[:, j, :], kg(g)[:, j * 128:(j + 1) * 128],
                                     qg(g)[:, 0:256], start=True, stop=True)
                    nc.tensor.matmul(psb[:, j, :], kg(g)[:, j * 128:(j + 1) * 128],
                                     qg(g)[:, 256:QW], start=True, stop=True)
                nc.scalar.copy(ssb[:, g, :, 0:256], psa)
                nc.scalar.copy(ssb[:, g, :, 256:QW], psb)

            osb = wpool.tile([64, NH, QW], F32, tag="osb")
            zsb = wpool.tile([65, NH, QW], F32, tag="zsb")
            zrec = wpool.tile([65, NH, QW], F32, tag="zrec")
            for hh in range(NH):
                at = apool.tile([128, NJ, QW], BF, tag="at")
                tt = apool.tile([128, NJ, QW], BF, tag="tt")
                nc.vector.tensor_mul(at[:], ssb[:, 0], mwsb[:, 0 * NH + hh])
                for g in (1, 2):
                    nc.vector.tensor_mul(tt[:], ssb[:, g], mwsb[:, g * NH + hh])
                    nc.vector.tensor_add(at[:], at[:], tt[:])
                e = apool.tile([128, NJ, QW], BF, tag="e")
                nc.scalar.activation(e[:, 0:4], at[:, 0:4], AF.Exp)
                nc.scalar.activation(e[:, 4:5], at[:, 4:5], AF.Exp, bias=eb_s[:, 0:1])

                pov = ppO.tile([VW, QW], F32, tag="ov", name="psO1")
                for j in range(NJ):
                    nc.tensor.matmul(pov, vtb[:, j, hh * VW:(hh + 1) * VW], e[:, j, :],
                                     start=(j == 0), stop=(j == NJ - 1))
                nc.scalar.copy(osb[:, hh], pov[0:64])
                nc.scalar.copy(zsb[64:65, hh], pov[64:65])

            nc.vector.reciprocal(zrec[64:65], zsb[64:65])
            zrep = wpool.tile([64, NH, QW], F32, tag="zrep")
            nc.sync.dma_start(
                zrep[:], zrec[64:65, None, :, :].to_broadcast((1, 64, NH, QW)))
            on = wpool.tile([64, NH, QW], BF, tag="on")
            nc.vector.tensor_mul(on[:], osb[:], zrep[:])

            for q0, q1 in QCHUNKS:
                outsb = wpool.tile([128, C], BF, tag="outsb")
                for n0 in range(0, C, 256):
                    ps = ppP.tile([128, 256], F32, tag="prj", name="psP")[: q1 - q0, :]
                    for hh in range(NH):
                        nc.tensor.matmul(ps, on[:, hh, q0:q1], pw_s[:, hh, n0:n0 + 256],
                                         start=(hh == 0), stop=(hh == NH - 1))
                    nc.scalar.copy(outsb[: q1 - q0, n0:n0 + 256], ps)
                nc.sync.dma_start(out.ap()[b, q0:q1, :], outsb[: q1 - q0, :])

        # software pipeline: emit batch b+1's projections before batch b's
        # attention so the PE never stalls behind the ACT/DVE backlog
        prev = phase_a(0)
        for b in range(B):
            nxt = phase_a(b + 1) if b + 1 < B else None
            phase_b(b, *prev)
            prev = nxt

    nc.compile()
    return nc


def prep_inputs2(x, masks, Wq, Wk, Wv, mask_proj, proj_w, proj_b):
    """Returns (in_maps list for 8 cores, host-side finisher info)."""
    f16 = np.float16

    xhatT = np.zeros((B, CP, NP), np.float32)
    xhatT[:, :C, :N] = x.transpose(0, 2, 1)
    xhatT[:, C, :N] = 1.0
    xta = np.ascontiguousarray(
        xhatT.reshape(B, KO, 128, NP).transpose(0, 2, 1, 3)).astype(f16)

    def wpad(w, scale=1.0):
        wp = np.zeros((CP, w.shape[1]), np.float32)
        wp[:C] = w * scale
        return np.ascontiguousarray(wp.reshape(KO, 128, -1).transpose(1, 0, 2)).astype(f16)

    wqp = wpad(Wq, SCALE)
    wkp = wpad(Wk)

    mk = masks.transpose(2, 1, 0)  # [m, k, q]
    mkp = np.zeros((ML, NP, NP), np.float32)
    mkp[:, :N, :N] = mk
    mt_full = mkp.reshape(ML, NJ, 128, NP).transpose(2, 0, 1, 3).reshape(
        128, ML * NJ, NP).astype(f16)

    ebp = np.zeros((128, 1), np.float32)
    ebp[65:, 0] = EXP_NEG

    in_maps = []
    for c in range(8):
        hg, s = c // 2, c % 2
        H0 = NH * hg
        qo = QW * s

        wvh = np.zeros((CP, NH * VW), np.float32)
        for hh in range(NH):
            h = H0 + hh
            wvh[:C, hh * VW:hh * VW + HD] = Wv[:, h * HD:(h + 1) * HD]
            wvh[C, hh * VW + HD] = 1.0
        wvp = np.ascontiguousarray(
            wvh.reshape(KO, 128, -1).transpose(1, 0, 2)).astype(f16)

        pwp = np.ascontiguousarray(
            proj_w.reshape(LH, 64, C)[H0:H0 + NH].transpose(1, 0, 2)).astype(f16)

        cwv = np.zeros(GH * ML * NH, np.float32)
        for g in range(GH):
            for m in range(ML):
                for hh in range(NH):
                    cwv[(g * ML + m) * NH + hh] = mask_proj[m, g * LH + H0 + hh]
        cwp = np.broadcast_to(cwv, (128, GH * ML * NH)).copy()

        in_maps.append({
            "xta": xta,
            "xqa": np.ascontiguousarray(xta[:, :, :, qo:qo + QW]),
            "wq": wqp, "wk": wkp, "wv": wvp, "pw": pwp,
            "mt": np.ascontiguousarray(mt_full[:, :, qo:qo + QW]),
            "cw": cwp, "eb": ebp,
        })
    return in_maps


_NC2 = None


def get_nc2():
    global _NC2
    if _NC2 is None:
        _NC2 = build_nc2()
    return _NC2


def kernel_v2(x, masks, Wq, Wk, Wv, mask_proj, proj_w, proj_b):
    x = np.asarray(x, np.float32)
    in_maps = prep_inputs2(
        x, np.asarray(masks, np.float32), np.asarray(Wq, np.float32),
        np.asarray(Wk, np.float32), np.asarray(Wv, np.float32),
        np.asarray(mask_proj, np.float32), np.asarray(proj_w, np.float32),
        np.asarray(proj_b, np.float32))
    res = bass_utils.run_bass_kernel_spmd(get_nc2(), in_maps, core_ids=list(range(8)))
    # sum the 4 head-group partials per q-half, concat halves, add bias
    out = np.zeros((B, NP, C), np.float32)
    for c in range(8):
        hg, s = c // 2, c % 2
        out[:, QW * s:QW * (s + 1), :] += np.asarray(
            res.results[c]["op"], np.float32)
    out = out[:, :N, :] + np.asarray(proj_b, np.float32)
    return out.astype(np.float32)

def kernel(x, masks, Wq, Wk, Wv, mask_proj, proj_w, proj_b):
    import kernel_v3
    return kernel_v3.kernel_v3(x, masks, Wq, Wk, Wv, mask_proj, proj_w, proj_b)


if __name__ == "__main__":
    rng = np.random.default_rng(0)
    ins = {
        "x": rng.standard_normal((B, N, C)).astype(np.float32),
        "masks": rng.random((N, N, ML)).astype(np.float32),
        "Wq": (rng.standard_normal((C, GH * HD)) * 0.02).astype(np.float32),
        "Wk": (rng.standard_normal((C, GH * HD)) * 0.02).astype(np.float32),
        "Wv": (rng.standard_normal((C, C)) * 0.02).astype(np.float32),
        "mask_proj": (rng.standard_normal((ML, GH * LH)) * 0.5 + 1.0).astype(np.float32),
        "proj_w": (rng.standard_normal((C, C)) * 0.02).astype(np.float32),
        "proj_b": (rng.standard_normal(C) * 0.02).astype(np.float32),
    }
    out = kernel(**ins)
    print(out.shape, out.dtype)



# revision 3
# speedup vs baseline: 1.0433x; 1.0433x over previous
# Trainium2 Bass kernel for nn_Attention_48052094107920 (sparse_attention).
# See build_nc3 docstring comments below for the design.
# v3: batch-pair x query-half sharding with host-folded mask weights.
#
# Core c = (batch-pair p = c//2, q-half s = c%2). Each core processes its 2
# batches for ALL 12 local heads over a 289-column query slice (s=0 covers
# q 0:289, s=1 covers q 288:577; the host drops the overlap column). Outputs
# are full projections (transposed layout); the host transposes, adds
# proj_b, and concatenates -- no cross-core reduction.
#
# Key design points vs v2:
#   - mw[g,h] = sum_m mask_proj[m, g*12+h] * masks_m is computed on the HOST
#     (batch-independent weight folding) and streamed per-head from DRAM
#     (13.3MB/core, overlapped with the head loop under the DMA-device
#     budget). Kills the on-chip DVE precompute entirely.
#   - mix at_h = sum_g S_g * mw[g,h]: products for g=1,2 on the Pool
#     (gpsimd) engine (otherwise idle, tensor ops at 1.2GHz), the g=0
#     product and both adds on DVE (fp16 2x mode).
#   - exp: ONE activation per (b,h): padded k rows have S=0 (zero-padded x)
#     and mw=0 (host zeros) -> at=0 -> e=1, and vhat rows there are 0
#     (including the ones-column that generates Z), so pads contribute
#     nothing to p@v or Z. No pad-bias pass.
#   - v projection contracts only the real 768 channels (6 steps); the
#     per-head ones-columns that generate Z during p@v come from a tiny
#     host-provided token-validity mask copied in by the Pool engine.
#   - output projection emitted transposed ([c-chunk, q]): the contraction
#     runs as 12 64-deep steps costing q-width per step, and the lhsT is the
#     proj weight directly; host undoes the transpose.
#   - head loop interleaves the two batches with batch 0 running 3 heads
#     ahead, so batch 1's phase A overlaps batch 0's first heads and every
#     in-order engine queue stays busy; mw tiles rotate through 4 buffers.
#   - softmax normalization per 4-head group: in-place reciprocal on the Z
#     row of the o/Z staging tile, DMA partition-broadcast, one multiply.

import numpy as np

import concourse.bass as bass
import concourse.bacc as bacc_mod
import concourse.mybir as mybir
import concourse.tile as tile
from concourse import bass_utils

BF = mybir.dt.float16
F32 = mybir.dt.float32
AF = mybir.ActivationFunctionType
OP = mybir.AluOpType

B, N, C = 8, 577, 768
GH, LH, ML, HD = 3, 12, 3, 64
SCALE = HD ** -0.5
NP = 640          # padded tokens (5 * 128)
NJ = 5            # k chunks of 128
KQ = 6            # contraction chunks (768 channels)
QW = 289          # query-half width (s=0: 0:289, s=1: 288:577)
VW = HD + 1       # 65: per-head v block [v | ones]


def build_nc3():
    nc = bacc_mod.Bacc("TRN2", target_bir_lowering=False, debug=False, num_devices=8)

    xt = nc.dram_tensor("xt", [128, 2, KQ, NP], BF, kind="ExternalInput")
    xq = nc.dram_tensor("xq", [128, 2, KQ, QW], BF, kind="ExternalInput")
    wq = nc.dram_tensor("wq", [128, KQ, GH * HD], BF, kind="ExternalInput")
    wk = nc.dram_tensor("wk", [128, KQ, GH * HD], BF, kind="ExternalInput")
    wv = nc.dram_tensor("wv", [128, KQ, LH * VW], BF, kind="ExternalInput")
    vm = nc.dram_tensor("vm", [128, NJ, LH], BF, kind="ExternalInput")
    pw = nc.dram_tensor("pw", [64, LH, C], BF, kind="ExternalInput")
    mw = nc.dram_tensor("mw", [128, LH, GH, NJ, QW], BF, kind="ExternalInput")
    out = nc.dram_tensor("ot", [2, 6, 128, QW], BF, kind="ExternalOutput")

    with tile.TileContext(nc) as tc, \
         tc.tile_pool(name="const", bufs=1) as cpool, \
         tc.tile_pool(name="mwst", bufs=4) as mpool, \
         tc.tile_pool(name="work", bufs=2) as wpool, \
         tc.tile_pool(name="atp", bufs=3) as atpool, \
         tc.tile_pool(name="ttp", bufs=2) as ttpool, \
         tc.tile_pool(name="ep", bufs=5) as epool, \
         tc.tile_pool(name="psA", bufs=2, space="PSUM") as ppA, \
         tc.tile_pool(name="psS", bufs=3, space="PSUM") as ppS, \
         tc.tile_pool(name="psO", bufs=2, space="PSUM") as ppO, \
         tc.tile_pool(name="psZ", bufs=1, space="PSUM") as ppZ:

        # ---- input loads, ordered so compute starts ASAP ----
        wq_s = cpool.tile([128, KQ, GH * HD], BF, tag="wq")
        nc.sync.dma_start(wq_s[:], wq.ap())
        xq_s = cpool.tile([128, 2, KQ, QW], BF, tag="xq")
        nc.sync.dma_start(xq_s[:, 0], xq.ap()[:, 0])
        wk_s = cpool.tile([128, KQ, GH * HD], BF, tag="wk")
        nc.sync.dma_start(wk_s[:], wk.ap())
        xt_s = cpool.tile([128, 2, KQ, NP], BF, tag="xt")
        nc.sync.dma_start(xt_s[:, 0], xt.ap()[:, 0])
        def mw_load(h):
            t = mpool.tile([128, GH, NJ, QW], BF, tag="mwh", name=f"mw{h}")
            nc.sync.dma_start(t[:], mw.ap()[:, h])
            return t

        mwq = {0: mw_load(0)}
        wv_s = cpool.tile([128, KQ, LH * VW], BF, tag="wv")
        nc.sync.dma_start(wv_s[:], wv.ap())
        vm_s = cpool.tile([128, NJ, LH], BF, tag="vm")
        nc.sync.dma_start(vm_s[:], vm.ap())
        mwq[1] = mw_load(1)
        nc.sync.dma_start(xq_s[:, 1], xq.ap()[:, 1])
        nc.sync.dma_start(xt_s[:, 1], xt.ap()[:, 1])
        mwq[2] = mw_load(2)
        mwq[3] = mw_load(3)

        # persistent per-batch state
        qT01 = [cpool.tile([128, QW], BF, tag=f"q01_{b}", name=f"q01_{b}") for b in range(2)]
        qT2 = [cpool.tile([64, QW], BF, tag=f"q2_{b}", name=f"q2_{b}") for b in range(2)]
        kT01 = [cpool.tile([128, NP], BF, tag=f"k01_{b}", name=f"k01_{b}") for b in range(2)]
        kT2 = [cpool.tile([64, NP], BF, tag=f"k2_{b}", name=f"k2_{b}") for b in range(2)]
        vt = [cpool.tile([128, NJ, LH * VW], BF, tag=f"vt_{b}", name=f"vt_{b}") for b in range(2)]
        ssb = [cpool.tile([128, GH, NJ, QW], BF, tag=f"ssb_{b}", name=f"ssb_{b}") for b in range(2)]
        povs = [cpool.tile([VW, LH, QW], BF, tag=f"pov_{b}", name=f"pov_{b}") for b in range(2)]
        on = [cpool.tile([64, LH, QW], BF, tag=f"on_{b}", name=f"on_{b}") for b in range(2)]
        pw_box = [None]
        ones_t = cpool.tile([VW, 64], BF, tag="ones")
        nc.vector.memset(ones_t[:], 1.0)

        def qk_proj(b):
            # q projection (289 cols), channels on partitions
            for msl, mp, dst in ((slice(0, 128), 128, qT01[b]),
                                 (slice(128, 192), 64, qT2[b])):
                ps = ppA.tile([128, 512], F32, tag="bigA", name="psA")[:mp, :QW]
                for o in range(KQ):
                    nc.tensor.matmul(ps, wq_s[:, o, msl], xq_s[:, b, o, :],
                                     start=(o == 0), stop=(o == KQ - 1))
                nc.scalar.copy(dst[:mp, :], ps)
            # k projection (full 640; padded tokens project to 0)
            for msl, mp, dst in ((slice(0, 128), 128, kT01[b]),
                                 (slice(128, 192), 64, kT2[b])):
                for n0, n1 in ((0, 512), (512, NP)):
                    ps = ppA.tile([128, 512], F32, tag="bigA", name="psA")[:mp, : n1 - n0]
                    for o in range(KQ):
                        nc.tensor.matmul(ps, wk_s[:, o, msl], xt_s[:, b, o, n0:n1],
                                         start=(o == 0), stop=(o == KQ - 1))
                    nc.scalar.copy(dst[:mp, n0:n1], ps)

        def v_proj(b, half):
            # v-hat projection for heads [6*half, 6*half+6): tokens on
            # partitions, interleaved [v_h | 0] blocks; the zero
            # ones-columns are then filled from vm
            n0, n1 = half * 6 * VW, (half + 1) * 6 * VW
            for kc in range(NJ):
                ps = ppA.tile([128, 512], F32, tag="bigA", name="psA")[:, : n1 - n0]
                for o in range(KQ):
                    nc.tensor.matmul(ps, xt_s[:, b, o, kc * 128:(kc + 1) * 128],
                                     wv_s[:, o, n0:n1],
                                     start=(o == 0), stop=(o == KQ - 1))
                nc.scalar.copy(vt[b][:, kc, n0:n1], ps)
                nc.gpsimd.tensor_copy(vt[b][:, kc, n0 + HD:n1:VW],
                                      vm_s[:, kc, half * 6:half * 6 + 6])

        def qg(b, g):
            return (qT01[b][0:64], qT01[b][64:128], qT2[b][0:64])[g]

        def kg(b, g):
            return (kT01[b][0:64], kT01[b][64:128], kT2[b][0:64])[g]

        def scores(b):
            for g in range(GH):
                for j in range(NJ):
                    ps = ppS.tile([128, QW], F32, tag="s", name="psS")
                    nc.tensor.matmul(ps, kg(b, g)[:, j * 128:(j + 1) * 128],
                                     qg(b, g), start=True, stop=True)
                    if b == 0:
                        nc.vector.tensor_copy(ssb[b][:, g, j], ps)
                    else:
                        nc.scalar.copy(ssb[b][:, g, j], ps)

        def head(b, h, mwt):
            at = atpool.tile([128, NJ, QW], BF, tag="at")
            tb = ttpool.tile([128, NJ, QW], BF, tag="tb")
            tt = ttpool.tile([128, NJ, QW], BF, tag="tt")
            nc.gpsimd.tensor_mul(tb[:], ssb[b][:, 1], mwt[:, 1])
            nc.gpsimd.tensor_mul(tt[:], ssb[b][:, 2], mwt[:, 2])
            nc.vector.tensor_mul(at[:], ssb[b][:, 0], mwt[:, 0])
            nc.vector.tensor_add(at[:], at[:], tb[:])
            nc.vector.tensor_add(at[:], at[:], tt[:])
            e = epool.tile([128, NJ, QW], BF, tag="e")
            nc.scalar.activation(e[:], at[:], AF.Exp)
            pov = ppO.tile([VW, QW], F32, tag="ov", name="psO")
            for j in range(NJ):
                nc.tensor.matmul(pov, vt[b][:, j, h * VW:(h + 1) * VW], e[:, j, :],
                                 start=(j == 0), stop=(j == NJ - 1))
            nc.scalar.copy(povs[b][:, h], pov)

        def tail4(b, h0):
            hs = slice(h0, h0 + 4)
            with nc.allow_low_precision(reason="Z scaled into f16 range; 2e-2 tol"):
                nc.vector.reciprocal(povs[b][64:65, hs], povs[b][64:65, hs])
            zrep = cpool.tile([64, 4, QW], BF, tag="zrep", name="zrep")
            nc.sync.dma_start(
                zrep[:], povs[b][64:65, None, hs, :].to_broadcast((1, 64, 4, QW)))
            nc.gpsimd.tensor_mul(on[b][:, hs], povs[b][0:64, hs], zrep[:])

        def tail1(b, h):
            # per-head low-latency variant for the final exposed group:
            # PE replicates the 1/Z row into PSUM, DVE applies it
            with nc.allow_low_precision(reason="Z scaled into f16 range; 2e-2 tol"):
                nc.vector.reciprocal(povs[b][64:65, h], povs[b][64:65, h])
            zr = ppZ.tile([64, QW], F32, tag="zr", name="psZ")
            nc.tensor.matmul(zr, ones_t[64:65, :], povs[b][64:65, h],
                             start=True, stop=True)
            nc.vector.tensor_mul(on[b][:, h], povs[b][0:64, h], zr)

        def proj(b):
            outsb = wpool.tile([128, 6, QW], BF, tag="outsb")
            pw_s = pw_box[0]
            for c0 in (0, 3):
                pss = [ppS.tile([128, QW], F32, tag="s", name=f"psP{c0+i}")
                       for i in range(3)]
                for h in range(LH):
                    for i, ps in enumerate(pss):
                        nc.tensor.matmul(
                            ps, pw_s[:, h, (c0 + i) * 128:(c0 + i + 1) * 128],
                            on[b][:, h, :],
                            start=(h == 0), stop=(h == LH - 1))
                for i, ps in enumerate(pss):
                    nc.scalar.copy(outsb[:, c0 + i], ps)
            nc.sync.dma_start(
                out.ap()[b].rearrange("c p q -> p c q"), outsb[:])

        # ---- schedule ----
        # PE warmup: tiny matmuls so the p-state ramp completes before the
        # real projection chains arrive
        for _ in range(36):
            zw = ppZ.tile([64, QW], F32, tag="zr", name="psZ")[:, 0:64]
            nc.tensor.matmul(zw, ones_t[64:65, :], ones_t[64:65, :],
                             start=True, stop=True)

        qk_proj(0)
        scores(0)
        v_proj(0, 0)

        def run_head(b, h):
            head(b, h, mwq[h])
            if b == 1 and h >= LH - 4:
                tail1(b, h)
            elif h % 4 == 3:
                tail4(b, h - 3)

        run_head(0, 0)
        run_head(0, 1)
        qk_proj(1)
        scores(1)
        v_proj(1, 0)
        for h in range(2, LH):
            run_head(0, h)
            if h == 11:
                proj(0)
            run_head(1, h - 2)
            if h == 3:
                v_proj(0, 1)
            if h == 4:
                v_proj(1, 1)
            if h + 2 < LH:
                mwq[h + 2] = mw_load(h + 2)
            if h == 9:
                pw_s = cpool.tile([64, LH, C], BF, tag="pw")
                nc.sync.dma_start(pw_s[:], pw.ap())
                pw_box[0] = pw_s
        # proj(1) with heads 0..10 accumulated in open psum chains while
        # head 11 finishes; only the last step + copies trail the loop.
        # 5 chunks get open chains (psS x3 + psA x2); chunk 5 runs whole.
        run_head(1, LH - 2)
        pw_s = pw_box[0]
        pss1 = [ppS.tile([128, QW], F32, tag="s", name=f"psQ{i}")
                for i in range(3)]
        pss1 += [ppA.tile([128, 512], F32, tag="bigA", name=f"psR{i}")[:, :QW]
                 for i in range(2)]
        for h in range(LH - 1):
            for cc, ps in enumerate(pss1):
                nc.tensor.matmul(ps, pw_s[:, h, cc * 128:(cc + 1) * 128],
                                 on[1][:, h, :],
                                 start=(h == 0), stop=False)
        run_head(1, LH - 1)
        outsb1 = wpool.tile([128, 6, QW], BF, tag="outsb")
        for cc, ps in enumerate(pss1):
            nc.tensor.matmul(ps, pw_s[:, LH - 1, cc * 128:(cc + 1) * 128],
                             on[1][:, LH - 1, :], start=False, stop=True)
            nc.scalar.copy(outsb1[:, cc], ps)
        ps = ppZ.tile([64, QW], F32, tag="zr", name="psZ")
        ps6a = ppO.tile([VW, QW], F32, tag="ov", name="psO")[0:64, :]
        for h in range(LH):
            nc.tensor.matmul(ps6a, pw_s[0:64, h, 640:704], on[1][:, h, :],
                             start=(h == 0), stop=(h == LH - 1))
        for h in range(LH):
            nc.tensor.matmul(ps, pw_s[0:64, h, 704:768], on[1][:, h, :],
                             start=(h == 0), stop=(h == LH - 1))
        nc.scalar.copy(outsb1[0:64, 5], ps6a)
        c5b = wpool.tile([64, QW], BF, tag="c5b")
        nc.scalar.copy(c5b[:], ps)
        nc.sync.dma_start(
            out.ap()[1].rearrange("c p q -> p c q")[0:64, 0:5, :], outsb1[0:64, 0:5])
        nc.sync.dma_start(
            out.ap()[1].rearrange("c p q -> p c q")[64:128, 0:5, :], outsb1[64:128, 0:5])
        nc.sync.dma_start(out.ap()[1][5, 0:64, :], outsb1[0:64, 5])
        nc.sync.dma_start(out.ap()[1][5, 64:128, :], c5b[:])

    nc.compile()
    return nc


def prep_inputs3(x, masks, Wq, Wk, Wv, mask_proj, proj_w, proj_b):
    """Build the 8 per-core input maps."""
    f16 = np.float16

    xhatT = np.zeros((B, C, NP), np.float32)
    xhatT[:, :, :N] = x.transpose(0, 2, 1)
    xta = np.ascontiguousarray(
        xhatT.reshape(B, KQ, 128, NP).transpose(0, 2, 1, 3)).astype(f16)

    def wpad(w, scale=1.0):
        return np.ascontiguousarray(
            (w * scale).reshape(KQ, 128, -1).transpose(1, 0, 2)).astype(f16)

    wqp = wpad(Wq, SCALE)
    wkp = wpad(Wk)

    # v weights interleaved per head as [v_h (64) | zero ones-col]
    wvh = np.zeros((C, LH * VW), np.float32)
    for h in range(LH):
        wvh[:, h * VW:h * VW + HD] = Wv[:, h * HD:(h + 1) * HD]
    wvp = wpad(wvh, 1.0 / 64.0)

    # token-validity mask -> the per-head ones columns of v-hat
    vmp = np.zeros((128, NJ, LH), np.float32)
    for j in range(NJ):
        lim = min(max(N - j * 128, 0), 128)
        vmp[:lim, j, :] = 1.0 / 64.0
    vmp = vmp.astype(f16)

    pwp = np.ascontiguousarray(
        proj_w.reshape(LH, 64, C).transpose(1, 0, 2)).astype(f16)

    # host-folded mask weights: [k, q, g, h] zero-padded in k
    mw_nn = (masks.reshape(-1, ML).astype(np.float64)
             @ mask_proj.astype(np.float64)).astype(np.float32)
    mw_nn = mw_nn.reshape(N, N, GH, LH)          # [q, k, g, h]
    mw_kq = np.zeros((NP, N, GH, LH), np.float32)
    mw_kq[:N] = mw_nn.transpose(1, 0, 2, 3)      # [k, q, g, h]
    mw_full = np.ascontiguousarray(
        mw_kq.reshape(NJ, 128, N, GH, LH).transpose(1, 4, 3, 0, 2)).astype(f16)

    in_maps = []
    for c in range(8):
        p, s = c // 2, c % 2
        qo = 288 * s
        bsl = slice(2 * p, 2 * p + 2)
        in_maps.append({
            "xt": np.ascontiguousarray(xta[bsl].transpose(1, 0, 2, 3)),
            "xq": np.ascontiguousarray(
                xta[bsl, :, :, qo:qo + QW].transpose(1, 0, 2, 3)),
            "wq": wqp, "wk": wkp, "wv": wvp, "vm": vmp, "pw": pwp,
            "mw": np.ascontiguousarray(mw_full[:, :, :, :, qo:qo + QW]),
        })
    return in_maps


_NC3 = None


def get_nc3():
    global _NC3
    if _NC3 is None:
        _NC3 = build_nc3()
    return _NC3


def kernel(x, masks, Wq, Wk, Wv, mask_proj, proj_w, proj_b):
    x = np.asarray(x, np.float32)
    proj_b = np.asarray(proj_b, np.float32)
    in_maps = prep_inputs3(
        x, np.asarray(masks, np.float32), np.asarray(Wq, np.float32),
        np.asarray(Wk, np.float32), np.asarray(Wv, np.float32),
        np.asarray(mask_proj, np.float32), np.asarray(proj_w, np.float32),
        proj_b)
    res = bass_utils.run_bass_kernel_spmd(get_nc3(), in_maps, core_ids=list(range(8)))
    out = np.zeros((B, N, C), np.float32)
    for c in range(8):
        p, s = c // 2, c % 2
        ot = np.asarray(res.results[c]["ot"], np.float32)  # [2, 6, 128, QW]
        ot = ot.reshape(2, C, QW).transpose(0, 2, 1)       # [2, QW, C]
        for i, b in enumerate(range(2 * p, 2 * p + 2)):
            if s == 0:
                out[b, 0:289] = ot[i]
            else:
                out[b, 289:577] = ot[i, 1:]
    return (out + proj_b).astype(np.float32)


# revision 4
# speedup vs baseline: 1.1047x; 1.0589x over previous
# Trainium2 Bass kernel for nn_Attention_48052094107920 (sparse_attention).
# See build_nc3 docstring comments below for the design.
# v3: batch-pair x query-half sharding with host-folded mask weights.
#
# Core c = (batch-pair p = c//2, q-half s = c%2). Each core processes its 2
# batches for ALL 12 local heads over a 289-column query slice (s=0 covers
# q 0:289, s=1 covers q 288:577; the host drops the overlap column). Outputs
# are full projections (transposed layout); the host transposes, adds
# proj_b, and concatenates -- no cross-core reduction.
#
# Key design points vs v2:
#   - mw[g,h] = sum_m mask_proj[m, g*12+h] * masks_m is computed on the HOST
#     (batch-independent weight folding) and streamed per-head from DRAM
#     (13.3MB/core, overlapped with the head loop under the DMA-device
#     budget). Kills the on-chip DVE precompute entirely.
#   - mix at_h = sum_g S_g * mw[g,h]: products for g=1,2 on the Pool
#     (gpsimd) engine (otherwise idle, tensor ops at 1.2GHz), the g=0
#     product and both adds on DVE (fp16 2x mode).
#   - exp: ONE activation per (b,h): padded k rows have S=0 (zero-padded x)
#     and mw=0 (host zeros) -> at=0 -> e=1, and vhat rows there are 0
#     (including the ones-column that generates Z), so pads contribute
#     nothing to p@v or Z. No pad-bias pass.
#   - v projection contracts only the real 768 channels (6 steps); the
#     per-head ones-columns that generate Z during p@v come from a tiny
#     host-provided token-validity mask copied in by the Pool engine.
#   - output projection emitted transposed ([c-chunk, q]): the contraction
#     runs as 12 64-deep steps costing q-width per step, and the lhsT is the
#     proj weight directly; host undoes the transpose.
#   - head loop interleaves the two batches with batch 0 running 3 heads
#     ahead, so batch 1's phase A overlaps batch 0's first heads and every
#     in-order engine queue stays busy; mw tiles rotate through 4 buffers.
#   - softmax normalization per 4-head group: in-place reciprocal on the Z
#     row of the o/Z staging tile, DMA partition-broadcast, one multiply.

import numpy as np

import concourse.bass as bass
import concourse.bacc as bacc_mod
import concourse.mybir as mybir
import concourse.tile as tile
from concourse import bass_utils

BF = mybir.dt.float16
F32 = mybir.dt.float32
AF = mybir.ActivationFunctionType
OP = mybir.AluOpType

B, N, C = 8, 577, 768
GH, LH, ML, HD = 3, 12, 3, 64
SCALE = HD ** -0.5
NP = 640          # padded tokens (5 * 128)
NJ = 5            # k chunks of 128
KQ = 6            # contraction chunks (768 channels)
QW = 289          # query-half width (s=0: 0:289, s=1: 288:577)
VW = HD + 1       # 65: per-head v block [v | ones]


def build_nc3():
    nc = bacc_mod.Bacc("TRN2", target_bir_lowering=False, debug=False, num_devices=8)

    xt = nc.dram_tensor("xt", [128, 2, KQ, NP], BF, kind="ExternalInput")
    xq = nc.dram_tensor("xq", [128, 2, KQ, QW], BF, kind="ExternalInput")
    wq = nc.dram_tensor("wq", [128, KQ, GH * HD], BF, kind="ExternalInput")
    wk = nc.dram_tensor("wk", [128, KQ, GH * HD], BF, kind="ExternalInput")
    wv = nc.dram_tensor("wv", [128, KQ, LH * VW], BF, kind="ExternalInput")
    vm = nc.dram_tensor("vm", [128, NJ, LH], BF, kind="ExternalInput")
    pw = nc.dram_tensor("pw", [128, 6, C], BF, kind="ExternalInput")
    mw = nc.dram_tensor("mw", [128, LH, GH, NJ, QW], BF, kind="ExternalInput")
    out = nc.dram_tensor("ot", [2, 6, 128, QW], BF, kind="ExternalOutput")

    with tile.TileContext(nc) as tc, \
         tc.tile_pool(name="const", bufs=1) as cpool, \
         tc.tile_pool(name="mwst", bufs=4) as mpool, \
         tc.tile_pool(name="work", bufs=2) as wpool, \
         tc.tile_pool(name="atp", bufs=3) as atpool, \
         tc.tile_pool(name="ttp", bufs=2) as ttpool, \
         tc.tile_pool(name="ep", bufs=5) as epool, \
         tc.tile_pool(name="psA", bufs=2, space="PSUM") as ppA, \
         tc.tile_pool(name="psS", bufs=3, space="PSUM") as ppS, \
         tc.tile_pool(name="psO", bufs=2, space="PSUM") as ppO, \
         tc.tile_pool(name="psZ", bufs=1, space="PSUM") as ppZ:

        # ---- input loads, ordered so compute starts ASAP ----
        wq_s = cpool.tile([128, KQ, GH * HD], BF, tag="wq")
        nc.sync.dma_start(wq_s[:], wq.ap())
        xq_s = cpool.tile([128, 2, KQ, QW], BF, tag="xq")
        nc.sync.dma_start(xq_s[:, 0], xq.ap()[:, 0])
        wk_s = cpool.tile([128, KQ, GH * HD], BF, tag="wk")
        nc.sync.dma_start(wk_s[:], wk.ap())
        xt_s = cpool.tile([128, 2, KQ, NP], BF, tag="xt")
        nc.sync.dma_start(xt_s[:, 0], xt.ap()[:, 0])
        def mw_load(h):
            t = mpool.tile([128, GH, NJ, QW], BF, tag="mwh", name=f"mw{h}")
            nc.sync.dma_start(t[:], mw.ap()[:, h])
            return t

        mwq = {0: mw_load(0)}
        wv_s = cpool.tile([128, KQ, LH * VW], BF, tag="wv")
        nc.sync.dma_start(wv_s[:], wv.ap())
        vm_s = cpool.tile([128, NJ, LH], BF, tag="vm")
        nc.sync.dma_start(vm_s[:], vm.ap())
        mwq[1] = mw_load(1)
        nc.sync.dma_start(xq_s[:, 1], xq.ap()[:, 1])
        nc.sync.dma_start(xt_s[:, 1], xt.ap()[:, 1])
        mwq[2] = mw_load(2)
        mwq[3] = mw_load(3)

        # persistent per-batch state
        qT01 = [cpool.tile([128, QW], BF, tag=f"q01_{b}", name=f"q01_{b}") for b in range(2)]
        qT2 = [cpool.tile([64, QW], BF, tag=f"q2_{b}", name=f"q2_{b}") for b in range(2)]
        kT01 = [cpool.tile([128, NP], BF, tag=f"k01_{b}", name=f"k01_{b}") for b in range(2)]
        kT2 = [cpool.tile([64, NP], BF, tag=f"k2_{b}", name=f"k2_{b}") for b in range(2)]
        vt = [cpool.tile([128, NJ, LH * VW], BF, tag=f"vt_{b}", name=f"vt_{b}") for b in range(2)]
        ssb = [cpool.tile([128, GH, NJ, QW], BF, tag=f"ssb_{b}", name=f"ssb_{b}") for b in range(2)]
        povs = [cpool.tile([VW, LH, QW], BF, tag=f"pov_{b}", name=f"pov_{b}") for b in range(2)]
        on = [[cpool.tile([128, 2, QW], BF, tag=f"on_{b}_{g}", name=f"on_{b}_{g}")
               for g in range(3)] for b in range(2)]
        pw_box = [None]
        ones_t = cpool.tile([VW, 64], BF, tag="ones")
        nc.vector.memset(ones_t[:], 1.0)

        def qk_proj(b):
            # q projection (289 cols), channels on partitions
            for msl, mp, dst in ((slice(0, 128), 128, qT01[b]),
                                 (slice(128, 192), 64, qT2[b])):
                ps = ppA.tile([128, 512], F32, tag="bigA", name="psA")[:mp, :QW]
                for o in range(KQ):
                    nc.tensor.matmul(ps, wq_s[:, o, msl], xq_s[:, b, o, :],
                                     start=(o == 0), stop=(o == KQ - 1))
                nc.scalar.copy(dst[:mp, :], ps)
            # k projection (full 640; padded tokens project to 0)
            for msl, mp, dst in ((slice(0, 128), 128, kT01[b]),
                                 (slice(128, 192), 64, kT2[b])):
                for n0, n1 in ((0, 512), (512, NP)):
                    ps = ppA.tile([128, 512], F32, tag="bigA", name="psA")[:mp, : n1 - n0]
                    for o in range(KQ):
                        nc.tensor.matmul(ps, wk_s[:, o, msl], xt_s[:, b, o, n0:n1],
                                         start=(o == 0), stop=(o == KQ - 1))
                    nc.scalar.copy(dst[:mp, n0:n1], ps)

        def v_proj(b, half):
            # v-hat projection for heads [6*half, 6*half+6): tokens on
            # partitions, interleaved [v_h | 0] blocks; the zero
            # ones-columns are then filled from vm
            n0, n1 = half * 6 * VW, (half + 1) * 6 * VW
            for kc in range(NJ):
                ps = ppA.tile([128, 512], F32, tag="bigA", name="psA")[:, : n1 - n0]
                for o in range(KQ):
                    nc.tensor.matmul(ps, xt_s[:, b, o, kc * 128:(kc + 1) * 128],
                                     wv_s[:, o, n0:n1],
                                     start=(o == 0), stop=(o == KQ - 1))
                nc.scalar.copy(vt[b][:, kc, n0:n1], ps)
                nc.gpsimd.tensor_copy(vt[b][:, kc, n0 + HD:n1:VW],
                                      vm_s[:, kc, half * 6:half * 6 + 6])

        def qg(b, g):
            return (qT01[b][0:64], qT01[b][64:128], qT2[b][0:64])[g]

        def kg(b, g):
            return (kT01[b][0:64], kT01[b][64:128], kT2[b][0:64])[g]

        def scores(b):
            for g in range(GH):
                for j in range(NJ):
                    ps = ppS.tile([128, QW], F32, tag="s", name="psS")
                    nc.tensor.matmul(ps, kg(b, g)[:, j * 128:(j + 1) * 128],
                                     qg(b, g), start=True, stop=True)
                    if b == 0:
                        nc.vector.tensor_copy(ssb[b][:, g, j], ps)
                    else:
                        nc.scalar.copy(ssb[b][:, g, j], ps)

        def head(b, h, mwt):
            at = atpool.tile([128, NJ, QW], BF, tag="at")
            tb = ttpool.tile([128, NJ, QW], BF, tag="tb")
            tt = ttpool.tile([128, NJ, QW], BF, tag="tt")
            nc.gpsimd.tensor_mul(tb[:], ssb[b][:, 1], mwt[:, 1])
            nc.gpsimd.tensor_mul(tt[:], ssb[b][:, 2], mwt[:, 2])
            nc.vector.tensor_mul(at[:], ssb[b][:, 0], mwt[:, 0])
            nc.vector.tensor_add(at[:], at[:], tb[:])
            nc.vector.tensor_add(at[:], at[:], tt[:])
            e = epool.tile([128, NJ, QW], BF, tag="e")
            nc.scalar.activation(e[:], at[:], AF.Exp)
            pov = ppO.tile([VW, QW], F32, tag="ov", name="psO")
            for j in range(NJ):
                nc.tensor.matmul(pov, vt[b][:, j, h * VW:(h + 1) * VW], e[:, j, :],
                                 start=(j == 0), stop=(j == NJ - 1))
            nc.scalar.copy(povs[b][:, h], pov)

        def tail4(b, h0):
            hs = slice(h0, h0 + 4)
            g = h0 // 4
            with nc.allow_low_precision(reason="Z scaled into f16 range; 2e-2 tol"):
                nc.vector.reciprocal(povs[b][64:65, hs], povs[b][64:65, hs])
            zrep = cpool.tile([64, 4, QW], BF, tag="zrep", name="zrep")
            nc.sync.dma_start(
                zrep[:], povs[b][64:65, None, hs, :].to_broadcast((1, 64, 4, QW)))
            nc.gpsimd.tensor_mul(on[b][g][0:64], povs[b][0:64, h0:h0 + 4:2],
                                 zrep[:, 0::2])
            ot_ = wpool.tile([64, 2, QW], BF, tag="otmp")
            nc.gpsimd.tensor_mul(ot_[:], povs[b][0:64, h0 + 1:h0 + 4:2],
                                 zrep[:, 1::2])
            nc.sync.dma_start(on[b][g][64:128], ot_[:])

        def tail1(b, h):
            # per-head low-latency variant for the final exposed group:
            # PE replicates the 1/Z row into PSUM, DVE applies it
            g, p, odd = h // 4, (h % 4) // 2, h % 2
            with nc.allow_low_precision(reason="Z scaled into f16 range; 2e-2 tol"):
                nc.vector.reciprocal(povs[b][64:65, h], povs[b][64:65, h])
            zr = ppO.tile([VW, QW], F32, tag="ov", name="psO")[0:64, :]
            nc.tensor.matmul(zr, ones_t[64:65, :], povs[b][64:65, h],
                             start=True, stop=True)
            if odd:
                o1 = wpool.tile([64, QW], BF, tag="otmp1")
                nc.vector.tensor_mul(o1[:], povs[b][0:64, h], zr)
                nc.sync.dma_start(on[b][g][64:128, p], o1[:])
            else:
                nc.vector.tensor_mul(on[b][g][0:64, p], povs[b][0:64, h], zr)

        def proj(b):
            outsb = wpool.tile([128, 6, QW], BF, tag="outsb")
            pw_s = pw_box[0]
            for c0 in (0, 3):
                pss = [ppS.tile([128, QW], F32, tag="s", name=f"psP{c0+i}")
                       for i in range(3)]
                for h in range(LH):
                    for i, ps in enumerate(pss):
                        nc.tensor.matmul(
                            ps, pw_s[:, h, (c0 + i) * 128:(c0 + i + 1) * 128],
                            on[b][:, h, :],
                            start=(h == 0), stop=(h == LH - 1))
                for i, ps in enumerate(pss):
                    nc.scalar.copy(outsb[:, c0 + i], ps)
            nc.sync.dma_start(
                out.ap()[b].rearrange("c p q -> p c q"), outsb[:])

        # ---- schedule ----
        # PE warmup: tiny matmuls so the p-state ramp completes before the
        # real projection chains arrive
        for _ in range(36):
            zw = ppZ.tile([128, QW], F32, tag="zr", name="psZ")[0:64, 0:64]
            nc.tensor.matmul(zw, ones_t[64:65, :], ones_t[64:65, :],
                             start=True, stop=True)

        qk_proj(0)
        scores(0)
        v_proj(0, 0)

        def run_head(b, h):
            head(b, h, mwq[h])
            if b == 1 and h >= LH - 4:
                tail1(b, h)
            elif h % 4 == 3:
                tail4(b, h - 3)

        run_head(0, 0)
        run_head(0, 1)
        qk_proj(1)
        scores(1)
        v_proj(1, 0)
        pss0 = [None]

        def open_steps(pss, b, p0, p1):
            # pair-steps: contraction over 128 = 2 heads x 64 dims
            pw_s = pw_box[0]
            for pp in range(p0, p1):
                for cc, ps in enumerate(pss):
                    nc.tensor.matmul(ps, pw_s[:, pp, cc * 128:(cc + 1) * 128],
                                     on[b][pp // 2][:, pp % 2, :],
                                     start=(pp == 0), stop=False)

        for h in range(2, LH):
            run_head(0, h)
            run_head(1, h - 2)
            if h == 2:
                pw_s = cpool.tile([128, 6, C], BF, tag="pw")
                nc.sync.dma_start(pw_s[:, 0:3], pw.ap()[:, 0:3])
                pw_box[0] = pw_s
            if h == 3:
                v_proj(0, 1)
                nc.sync.dma_start(pw_box[0][:, 3:6], pw.ap()[:, 3:6])
            if h == 4:
                v_proj(1, 1)
            if h + 2 < LH:
                mwq[h + 2] = mw_load(h + 2)
            if h == 8:
                pss0[0] = [ppS.tile([128, QW], F32, tag="s", name=f"psP{i}")
                           for i in range(3)]
                pss0[0] += [ppA.tile([128, 512], F32, tag="bigA",
                                     name=f"psPA{i}")[:, :QW] for i in range(2)]
                open_steps(pss0[0], 0, 0, 2)
            if h == 9:
                open_steps(pss0[0], 0, 2, 4)
        pw_s = pw_box[0]

        def close5(b, pss, outsb):
            # final 4 head-steps on the 5 open chains, then per-chunk copies
            # and immediate DMAs so the output drains as it lands
            for pp in range(4, 6):
                for cc, ps in enumerate(pss):
                    nc.tensor.matmul(ps, pw_s[:, pp, cc * 128:(cc + 1) * 128],
                                     on[b][pp // 2][:, pp % 2, :],
                                     start=False, stop=(pp == 5))
            for cc, ps in enumerate(pss):
                if cc % 2 == 0:
                    nc.scalar.copy(outsb[:, cc], ps)
                else:
                    nc.vector.tensor_copy(outsb[:, cc], ps)
                nc.sync.dma_start(out.ap()[b, cc], outsb[:, cc])

        def chunk5(b, outsb):
            # full-width chunk-5 chain on the psZ bank (zr lives in ppO now)
            ps6 = ppZ.tile([128, QW], F32, tag="zr", name="psZ")
            for pp in range(6):
                nc.tensor.matmul(ps6, pw_s[:, pp, 640:768],
                                 on[b][pp // 2][:, pp % 2, :],
                                 start=(pp == 0), stop=(pp == 5))
            nc.vector.tensor_copy(outsb[:, 5], ps6)
            nc.sync.dma_start(out.ap()[b, 5], outsb[:, 5])

        run_head(1, LH - 1)
        outsb0 = wpool.tile([128, 6, QW], BF, tag="outsb")
        close5(0, pss0[0], outsb0)
        chunk5(0, outsb0)
        pss1 = [ppS.tile([128, QW], F32, tag="s", name=f"psQ{i}")
                for i in range(3)]
        pss1 += [ppA.tile([128, 512], F32, tag="bigA", name=f"psR{i}")[:, :QW]
                 for i in range(2)]
        open_steps(pss1, 1, 0, 5)
        run_head(1, LH - 2)
        outsb1 = wpool.tile([128, 6, QW], BF, tag="outsb")
        for pp in (5,):
            for cc, ps in enumerate(pss1):
                nc.tensor.matmul(ps, pw_s[:, pp, cc * 128:(cc + 1) * 128],
                                 on[1][pp // 2][:, pp % 2, :],
                                 start=False, stop=True)
        for cc, ps in enumerate(pss1):
            if cc % 2 == 0:
                nc.scalar.copy(outsb1[:, cc], ps)
            else:
                nc.vector.tensor_copy(outsb1[:, cc], ps)
            nc.sync.dma_start(out.ap()[1, cc], outsb1[:, cc])
        chunk5(1, outsb1)

    nc.compile()
    return nc


def prep_inputs3(x, masks, Wq, Wk, Wv, mask_proj, proj_w, proj_b):
    """Build the 8 per-core input maps."""
    f16 = np.float16

    xhatT = np.zeros((B, C, NP), np.float32)
    xhatT[:, :, :N] = x.transpose(0, 2, 1)
    xta = np.ascontiguousarray(
        xhatT.reshape(B, KQ, 128, NP).transpose(0, 2, 1, 3)).astype(f16)

    def wpad(w, scale=1.0):
        return np.ascontiguousarray(
            (w * scale).reshape(KQ, 128, -1).transpose(1, 0, 2)).astype(f16)

    wqp = wpad(Wq, SCALE)
    wkp = wpad(Wk)

    # v weights interleaved per head as [v_h (64) | zero ones-col]
    wvh = np.zeros((C, LH * VW), np.float32)
    for h in range(LH):
        wvh[:, h * VW:h * VW + HD] = Wv[:, h * HD:(h + 1) * HD]
    wvp = wpad(wvh, 1.0 / 64.0)

    # token-validity mask -> the per-head ones columns of v-hat
    vmp = np.zeros((128, NJ, LH), np.float32)
    for j in range(NJ):
        lim = min(max(N - j * 128, 0), 128)
        vmp[:lim, j, :] = 1.0 / 64.0
    vmp = vmp.astype(f16)

    pwp = np.ascontiguousarray(
        proj_w.reshape(6, 2, 64, C).transpose(1, 2, 0, 3).reshape(128, 6, C)
    ).astype(f16)

    # host-folded mask weights: [k, q, g, h] zero-padded in k
    mw_nn = (masks.reshape(-1, ML).astype(np.float64)
             @ mask_proj.astype(np.float64)).astype(np.float32)
    mw_nn = mw_nn.reshape(N, N, GH, LH)          # [q, k, g, h]
    mw_kq = np.zeros((NP, N, GH, LH), np.float32)
    mw_kq[:N] = mw_nn.transpose(1, 0, 2, 3)      # [k, q, g, h]
    mw_full = np.ascontiguousarray(
        mw_kq.reshape(NJ, 128, N, GH, LH).transpose(1, 4, 3, 0, 2)).astype(f16)

    in_maps = []
    for c in range(8):
        p, s = c // 2, c % 2
        qo = 288 * s
        bsl = slice(2 * p, 2 * p + 2)
        in_maps.append({
            "xt": np.ascontiguousarray(xta[bsl].transpose(1, 0, 2, 3)),
            "xq": np.ascontiguousarray(
                xta[bsl, :, :, qo:qo + QW].transpose(1, 0, 2, 3)),
            "wq": wqp, "wk": wkp, "wv": wvp, "vm": vmp, "pw": pwp,
            "mw": np.ascontiguousarray(mw_full[:, :, :, :, qo:qo + QW]),
        })
    return in_maps


_NC3 = None


def get_nc3():
    global _NC3
    if _NC3 is None:
        _NC3 = build_nc3()
    return _NC3


def kernel(x, masks, Wq, Wk, Wv, mask_proj, proj_w, proj_b):
    x = np.asarray(x, np.float32)
    proj_b = np.asarray(proj_b, np.float32)
    in_maps = prep_inputs3(
        x, np.asarray(masks, np.float32), np.asarray(Wq, np.float32),
        np.asarray(Wk, np.float32), np.asarray(Wv, np.float32),
        np.asarray(mask_proj, np.float32), np.asarray(proj_w, np.float32),
        proj_b)
    res = bass_utils.run_bass_kernel_spmd(get_nc3(), in_maps, core_ids=list(range(8)))
    out = np.zeros((B, N, C), np.float32)
    for c in range(8):
        p, s = c // 2, c % 2
        ot = np.asarray(res.results[c]["ot"], np.float32)  # [2, 6, 128, QW]
        ot = ot.reshape(2, C, QW).transpose(0, 2, 1)       # [2, QW, C]
        for i, b in enumerate(range(2 * p, 2 * p + 2)):
            if s == 0:
                out[b, 0:289] = ot[i]
            else:
                out[b, 289:577] = ot[i, 1:]
    return (out + proj_b).astype(np.float32)


# revision 5
# speedup vs baseline: 1.1128x; 1.0073x over previous
# Trainium2 Bass kernel for nn_Attention_48052094107920 (sparse_attention).
# See build_nc3 docstring comments below for the design.
# v3: batch-pair x query-half sharding with host-folded mask weights.
#
# Core c = (batch-pair p = c//2, q-half s = c%2). Each core processes its 2
# batches for ALL 12 local heads over a 289-column query slice (s=0 covers
# q 0:289, s=1 covers q 288:577; the host drops the overlap column). Outputs
# are full projections (transposed layout); the host transposes, adds
# proj_b, and concatenates -- no cross-core reduction.
#
# Key design points vs v2:
#   - mw[g,h] = sum_m mask_proj[m, g*12+h] * masks_m is computed on the HOST
#     (batch-independent weight folding) and streamed per-head from DRAM
#     (13.3MB/core, overlapped with the head loop under the DMA-device
#     budget). Kills the on-chip DVE precompute entirely.
#   - mix at_h = sum_g S_g * mw[g,h]: products for g=1,2 on the Pool
#     (gpsimd) engine (otherwise idle, tensor ops at 1.2GHz), the g=0
#     product and both adds on DVE (fp16 2x mode).
#   - exp: ONE activation per (b,h): padded k rows have S=0 (zero-padded x)
#     and mw=0 (host zeros) -> at=0 -> e=1, and vhat rows there are 0
#     (including the ones-column that generates Z), so pads contribute
#     nothing to p@v or Z. No pad-bias pass.
#   - v projection contracts only the real 768 channels (6 steps); the
#     per-head ones-columns that generate Z during p@v come from a tiny
#     host-provided token-validity mask copied in by the Pool engine.
#   - output projection emitted transposed ([c-chunk, q]): the contraction
#     runs as 12 64-deep steps costing q-width per step, and the lhsT is the
#     proj weight directly; host undoes the transpose.
#   - head loop interleaves the two batches with batch 0 running 3 heads
#     ahead, so batch 1's phase A overlaps batch 0's first heads and every
#     in-order engine queue stays busy; mw tiles rotate through 4 buffers.
#   - softmax normalization per 4-head group: in-place reciprocal on the Z
#     row of the o/Z staging tile, DMA partition-broadcast, one multiply.

import numpy as np

import concourse.bass as bass
import concourse.bacc as bacc_mod
import concourse.mybir as mybir
import concourse.tile as tile
from concourse import bass_utils

BF = mybir.dt.float16
F32 = mybir.dt.float32
AF = mybir.ActivationFunctionType
OP = mybir.AluOpType

B, N, C = 8, 577, 768
GH, LH, ML, HD = 3, 12, 3, 64
SCALE = HD ** -0.5
NP = 640          # padded tokens (5 * 128)
NJ = 5            # k chunks of 128
KQ = 6            # contraction chunks (768 channels)
QW = 289          # query-half width (s=0: 0:289, s=1: 288:577)
VW = HD + 1       # 65: per-head v block [v | ones]


def build_nc3():
    nc = bacc_mod.Bacc("TRN2", target_bir_lowering=False, debug=False, num_devices=8)

    xt = nc.dram_tensor("xt", [128, 2, KQ, NP], BF, kind="ExternalInput")
    xq = nc.dram_tensor("xq", [128, 2, KQ, QW], BF, kind="ExternalInput")
    wq = nc.dram_tensor("wq", [128, KQ, GH * HD], BF, kind="ExternalInput")
    wk = nc.dram_tensor("wk", [128, KQ, GH * HD], BF, kind="ExternalInput")
    wv = nc.dram_tensor("wv", [128, KQ, LH * VW], BF, kind="ExternalInput")
    vm = nc.dram_tensor("vm", [128, NJ, LH], BF, kind="ExternalInput")
    pw = nc.dram_tensor("pw", [128, 6, C], BF, kind="ExternalInput")
    mw = nc.dram_tensor("mw", [128, LH, GH, NJ, QW], BF, kind="ExternalInput")
    out = nc.dram_tensor("ot", [2, 6, 128, QW], BF, kind="ExternalOutput")

    with tile.TileContext(nc) as tc, \
         tc.tile_pool(name="const", bufs=1) as cpool, \
         tc.tile_pool(name="mwst", bufs=4) as mpool, \
         tc.tile_pool(name="work", bufs=2) as wpool, \
         tc.tile_pool(name="atp", bufs=3) as atpool, \
         tc.tile_pool(name="ttp", bufs=2) as ttpool, \
         tc.tile_pool(name="ep", bufs=5) as epool, \
         tc.tile_pool(name="psA", bufs=2, space="PSUM") as ppA, \
         tc.tile_pool(name="psS", bufs=3, space="PSUM") as ppS, \
         tc.tile_pool(name="psO", bufs=2, space="PSUM") as ppO, \
         tc.tile_pool(name="psZ", bufs=1, space="PSUM") as ppZ:

        # ---- input loads, ordered so compute starts ASAP ----
        wq_s = cpool.tile([128, KQ, GH * HD], BF, tag="wq")
        nc.sync.dma_start(wq_s[:], wq.ap())
        xq_s = cpool.tile([128, 2, KQ, QW], BF, tag="xq")
        nc.sync.dma_start(xq_s[:, 0], xq.ap()[:, 0])
        wk_s = cpool.tile([128, KQ, GH * HD], BF, tag="wk")
        nc.sync.dma_start(wk_s[:], wk.ap())
        xt_s = cpool.tile([128, 2, KQ, NP], BF, tag="xt")
        nc.sync.dma_start(xt_s[:, 0], xt.ap()[:, 0])
        def mw_load(h):
            t = mpool.tile([128, GH, NJ, QW], BF, tag="mwh", name=f"mw{h}")
            nc.sync.dma_start(t[:], mw.ap()[:, h])
            return t

        mwq = {0: mw_load(0)}
        wv_s = cpool.tile([128, KQ, LH * VW], BF, tag="wv")
        nc.sync.dma_start(wv_s[:], wv.ap())
        vm_s = cpool.tile([128, NJ, LH], BF, tag="vm")
        nc.sync.dma_start(vm_s[:], vm.ap())
        mwq[1] = mw_load(1)
        nc.sync.dma_start(xq_s[:, 1], xq.ap()[:, 1])
        nc.sync.dma_start(xt_s[:, 1], xt.ap()[:, 1])
        mwq[2] = mw_load(2)
        mwq[3] = mw_load(3)

        # persistent per-batch state
        qT01 = [cpool.tile([128, QW], BF, tag=f"q01_{b}", name=f"q01_{b}") for b in range(2)]
        qT2 = [cpool.tile([64, QW], BF, tag=f"q2_{b}", name=f"q2_{b}") for b in range(2)]
        kT01 = [cpool.tile([128, NP], BF, tag=f"k01_{b}", name=f"k01_{b}") for b in range(2)]
        kT2 = [cpool.tile([64, NP], BF, tag=f"k2_{b}", name=f"k2_{b}") for b in range(2)]
        vt = [cpool.tile([128, NJ, LH * VW], BF, tag=f"vt_{b}", name=f"vt_{b}") for b in range(2)]
        ssb = [cpool.tile([128, GH, NJ, QW], BF, tag=f"ssb_{b}", name=f"ssb_{b}") for b in range(2)]
        povs = [cpool.tile([VW, LH, QW], BF, tag=f"pov_{b}", name=f"pov_{b}") for b in range(2)]
        on = [[cpool.tile([128, 2, QW], BF, tag=f"on_{b}_{g}", name=f"on_{b}_{g}")
               for g in range(3)] for b in range(2)]
        pw_box = [None]
        ones_t = cpool.tile([VW, 64], BF, tag="ones")
        nc.vector.memset(ones_t[:], 1.0)

        def qk_proj(b):
            cp = nc.scalar.copy if b == 0 else nc.vector.tensor_copy
            # q projection (289 cols), channels on partitions
            for msl, mp, dst in ((slice(0, 128), 128, qT01[b]),
                                 (slice(128, 192), 64, qT2[b])):
                ps = ppA.tile([128, 512], F32, tag="bigA", name="psA")[:mp, :QW]
                for o in range(KQ):
                    nc.tensor.matmul(ps, wq_s[:, o, msl], xq_s[:, b, o, :],
                                     start=(o == 0), stop=(o == KQ - 1))
                cp(dst[:mp, :], ps)
            # k projection (full 640; padded tokens project to 0)
            for msl, mp, dst in ((slice(0, 128), 128, kT01[b]),
                                 (slice(128, 192), 64, kT2[b])):
                for n0, n1 in ((0, 512), (512, NP)):
                    ps = ppA.tile([128, 512], F32, tag="bigA", name="psA")[:mp, : n1 - n0]
                    for o in range(KQ):
                        nc.tensor.matmul(ps, wk_s[:, o, msl], xt_s[:, b, o, n0:n1],
                                         start=(o == 0), stop=(o == KQ - 1))
                    cp(dst[:mp, n0:n1], ps)

        def v_proj(b, half):
            # v-hat projection for heads [6*half, 6*half+6): tokens on
            # partitions, interleaved [v_h | 0] blocks; the zero
            # ones-columns are then filled from vm
            cp = nc.scalar.copy
            n0, n1 = half * 6 * VW, (half + 1) * 6 * VW
            for kc in range(NJ):
                ps = ppA.tile([128, 512], F32, tag="bigA", name="psA")[:, : n1 - n0]
                for o in range(KQ):
                    nc.tensor.matmul(ps, xt_s[:, b, o, kc * 128:(kc + 1) * 128],
                                     wv_s[:, o, n0:n1],
                                     start=(o == 0), stop=(o == KQ - 1))
                cp(vt[b][:, kc, n0:n1], ps)
                nc.gpsimd.tensor_copy(vt[b][:, kc, n0 + HD:n1:VW],
                                      vm_s[:, kc, half * 6:half * 6 + 6])

        def qg(b, g):
            return (qT01[b][0:64], qT01[b][64:128], qT2[b][0:64])[g]

        def kg(b, g):
            return (kT01[b][0:64], kT01[b][64:128], kT2[b][0:64])[g]

        def scores(b):
            for g in range(GH):
                for j in range(NJ):
                    ps = ppS.tile([128, QW], F32, tag="s", name="psS")
                    nc.tensor.matmul(ps, kg(b, g)[:, j * 128:(j + 1) * 128],
                                     qg(b, g), start=True, stop=True)
                    if b == 0:
                        nc.vector.tensor_copy(ssb[b][:, g, j], ps)
                    else:
                        nc.scalar.copy(ssb[b][:, g, j], ps)

        def head(b, h, mwt):
            at = atpool.tile([128, NJ, QW], BF, tag="at")
            tb = ttpool.tile([128, NJ, QW], BF, tag="tb")
            tt = ttpool.tile([128, NJ, QW], BF, tag="tt")
            nc.gpsimd.tensor_mul(tb[:], ssb[b][:, 1], mwt[:, 1])
            nc.gpsimd.tensor_mul(tt[:], ssb[b][:, 2], mwt[:, 2])
            nc.vector.tensor_mul(at[:], ssb[b][:, 0], mwt[:, 0])
            nc.vector.tensor_add(at[:], at[:], tb[:])
            nc.vector.tensor_add(at[:], at[:], tt[:])
            e = epool.tile([128, NJ, QW], BF, tag="e")
            nc.scalar.activation(e[:], at[:], AF.Exp)
            pov = ppO.tile([VW, QW], F32, tag="ov", name="psO")
            for j in range(NJ):
                nc.tensor.matmul(pov, vt[b][:, j, h * VW:(h + 1) * VW], e[:, j, :],
                                 start=(j == 0), stop=(j == NJ - 1))
            nc.scalar.copy(povs[b][:, h], pov)

        def tail4(b, h0):
            hs = slice(h0, h0 + 4)
            g = h0 // 4
            with nc.allow_low_precision(reason="Z scaled into f16 range; 2e-2 tol"):
                nc.vector.reciprocal(povs[b][64:65, hs], povs[b][64:65, hs])
            zrep = cpool.tile([64, 4, QW], BF, tag="zrep", name="zrep")
            nc.sync.dma_start(
                zrep[:], povs[b][64:65, None, hs, :].to_broadcast((1, 64, 4, QW)))
            nc.gpsimd.tensor_mul(on[b][g][0:64], povs[b][0:64, h0:h0 + 4:2],
                                 zrep[:, 0::2])
            ot_ = wpool.tile([64, 2, QW], BF, tag="otmp")
            nc.gpsimd.tensor_mul(ot_[:], povs[b][0:64, h0 + 1:h0 + 4:2],
                                 zrep[:, 1::2])
            nc.sync.dma_start(on[b][g][64:128], ot_[:])

        def tail1(b, h):
            # per-head low-latency variant for the final exposed group:
            # PE replicates the 1/Z row into PSUM, DVE applies it
            g, p, odd = h // 4, (h % 4) // 2, h % 2
            with nc.allow_low_precision(reason="Z scaled into f16 range; 2e-2 tol"):
                nc.vector.reciprocal(povs[b][64:65, h], povs[b][64:65, h])
            zr = ppO.tile([VW, QW], F32, tag="ov", name="psO")[0:64, :]
            nc.tensor.matmul(zr, ones_t[64:65, :], povs[b][64:65, h],
                             start=True, stop=True)
            if odd:
                o1 = wpool.tile([64, QW], BF, tag="otmp1")
                nc.vector.tensor_mul(o1[:], povs[b][0:64, h], zr)
                nc.sync.dma_start(on[b][g][64:128, p], o1[:])
            else:
                nc.vector.tensor_mul(on[b][g][0:64, p], povs[b][0:64, h], zr)

        def proj(b):
            outsb = wpool.tile([128, 6, QW], BF, tag="outsb")
            pw_s = pw_box[0]
            for c0 in (0, 3):
                pss = [ppS.tile([128, QW], F32, tag="s", name=f"psP{c0+i}")
                       for i in range(3)]
                for h in range(LH):
                    for i, ps in enumerate(pss):
                        nc.tensor.matmul(
                            ps, pw_s[:, h, (c0 + i) * 128:(c0 + i + 1) * 128],
                            on[b][:, h, :],
                            start=(h == 0), stop=(h == LH - 1))
                for i, ps in enumerate(pss):
                    nc.scalar.copy(outsb[:, c0 + i], ps)
            nc.sync.dma_start(
                out.ap()[b].rearrange("c p q -> p c q"), outsb[:])

        # ---- schedule ----
        # PE warmup: tiny matmuls so the p-state ramp completes before the
        # real projection chains arrive
        for _ in range(36):
            zw = ppZ.tile([128, QW], F32, tag="zr", name="psZ")[0:64, 0:64]
            nc.tensor.matmul(zw, ones_t[64:65, :], ones_t[64:65, :],
                             start=True, stop=True)

        qk_proj(0)
        scores(0)
        v_proj(0, 0)

        def run_head(b, h):
            head(b, h, mwq[h])
            if b == 1 and h >= LH - 4:
                tail1(b, h)
            elif h % 4 == 3:
                tail4(b, h - 3)

        run_head(0, 0)
        run_head(0, 1)
        qk_proj(1)
        scores(1)
        v_proj(1, 0)
        pss0 = [None]
        outsb_box = [None]

        def close5(b, pss, outsb):
            # final 4 head-steps on the 5 open chains, then per-chunk copies
            # and immediate DMAs so the output drains as it lands
            pw_s = pw_box[0]
            for pp in range(4, 6):
                for cc, ps in enumerate(pss):
                    nc.tensor.matmul(ps, pw_s[:, pp, cc * 128:(cc + 1) * 128],
                                     on[b][pp // 2][:, pp % 2, :],
                                     start=False, stop=(pp == 5))
            for cc, ps in enumerate(pss):
                if cc % 2 == 0:
                    nc.scalar.copy(outsb[:, cc], ps)
                else:
                    nc.vector.tensor_copy(outsb[:, cc], ps)
                nc.sync.dma_start(out.ap()[b, cc], outsb[:, cc])

        def chunk5(b, outsb):
            # full-width chunk-5 chain on the psZ bank (zr lives in ppO now)
            pw_s = pw_box[0]
            ps6 = ppZ.tile([128, QW], F32, tag="zr", name="psZ")
            for pp in range(6):
                nc.tensor.matmul(ps6, pw_s[:, pp, 640:768],
                                 on[b][pp // 2][:, pp % 2, :],
                                 start=(pp == 0), stop=(pp == 5))
            nc.vector.tensor_copy(outsb[:, 5], ps6)
            nc.sync.dma_start(out.ap()[b, 5], outsb[:, 5])

        def open_steps(pss, b, p0, p1):
            # pair-steps: contraction over 128 = 2 heads x 64 dims
            pw_s = pw_box[0]
            for pp in range(p0, p1):
                for cc, ps in enumerate(pss):
                    nc.tensor.matmul(ps, pw_s[:, pp, cc * 128:(cc + 1) * 128],
                                     on[b][pp // 2][:, pp % 2, :],
                                     start=(pp == 0), stop=False)

        for h in range(2, LH):
            run_head(0, h)
            run_head(1, h - 2)
            if h == 2:
                pw_s = cpool.tile([128, 6, C], BF, tag="pw")
                nc.sync.dma_start(pw_s[:, 0:3], pw.ap()[:, 0:3])
                pw_box[0] = pw_s
            if h == 3:
                v_proj(0, 1)
                nc.sync.dma_start(pw_box[0][:, 3:6], pw.ap()[:, 3:6])
            if h == 4:
                v_proj(1, 1)
            if h + 2 < LH:
                mwq[h + 2] = mw_load(h + 2)
            if h == 8:
                pss0[0] = [ppS.tile([128, QW], F32, tag="s", name=f"psP{i}")
                           for i in range(3)]
                pss0[0] += [ppA.tile([128, 512], F32, tag="bigA",
                                     name=f"psPA{i}")[:, :QW] for i in range(2)]
                open_steps(pss0[0], 0, 0, 2)
            if h == 9:
                open_steps(pss0[0], 0, 2, 4)
            if h == 11:
                outsb_box[0] = wpool.tile([128, 6, QW], BF, tag="outsb", name="outsb0")
                chunk5(0, outsb_box[0])
        run_head(1, LH - 1)
        close5(0, pss0[0], outsb_box[0])
        pss1 = [ppS.tile([128, QW], F32, tag="s", name=f"psQ{i}")
                for i in range(3)]
        pss1 += [ppA.tile([128, 512], F32, tag="bigA", name=f"psR{i}")[:, :QW]
                 for i in range(2)]
        open_steps(pss1, 1, 0, 5)
        run_head(1, LH - 2)
        outsb1 = wpool.tile([128, 6, QW], BF, tag="outsb")
        for pp in (5,):
            for cc, ps in enumerate(pss1):
                nc.tensor.matmul(ps, pw_s[:, pp, cc * 128:(cc + 1) * 128],
                                 on[1][pp // 2][:, pp % 2, :],
                                 start=False, stop=True)
        for cc, ps in enumerate(pss1):
            if cc % 2 == 0:
                nc.scalar.copy(outsb1[:, cc], ps)
            else:
                nc.vector.tensor_copy(outsb1[:, cc], ps)
            nc.sync.dma_start(out.ap()[1, cc], outsb1[:, cc])
        chunk5(1, outsb1)

    nc.compile()
    return nc


def prep_inputs3(x, masks, Wq, Wk, Wv, mask_proj, proj_w, proj_b):
    """Build the 8 per-core input maps."""
    f16 = np.float16

    xhatT = np.zeros((B, C, NP), np.float32)
    xhatT[:, :, :N] = x.transpose(0, 2, 1)
    xta = np.ascontiguousarray(
        xhatT.reshape(B, KQ, 128, NP).transpose(0, 2, 1, 3)).astype(f16)

    def wpad(w, scale=1.0):
        return np.ascontiguousarray(
            (w * scale).reshape(KQ, 128, -1).transpose(1, 0, 2)).astype(f16)

    wqp = wpad(Wq, SCALE)
    wkp = wpad(Wk)

    # v weights interleaved per head as [v_h (64) | zero ones-col]
    wvh = np.zeros((C, LH * VW), np.float32)
    for h in range(LH):
        wvh[:, h * VW:h * VW + HD] = Wv[:, h * HD:(h + 1) * HD]
    wvp = wpad(wvh, 1.0 / 64.0)

    # token-validity mask -> the per-head ones columns of v-hat
    vmp = np.zeros((128, NJ, LH), np.float32)
    for j in range(NJ):
        lim = min(max(N - j * 128, 0), 128)
        vmp[:lim, j, :] = 1.0 / 64.0
    vmp = vmp.astype(f16)

    pwp = np.ascontiguousarray(
        proj_w.reshape(6, 2, 64, C).transpose(1, 2, 0, 3).reshape(128, 6, C)
    ).astype(f16)

    # host-folded mask weights: [k, q, g, h] zero-padded in k
    mw_nn = (masks.reshape(-1, ML).astype(np.float64)
             @ mask_proj.astype(np.float64)).astype(np.float32)
    mw_nn = mw_nn.reshape(N, N, GH, LH)          # [q, k, g, h]
    mw_kq = np.zeros((NP, N, GH, LH), np.float32)
    mw_kq[:N] = mw_nn.transpose(1, 0, 2, 3)      # [k, q, g, h]
    mw_full = np.ascontiguousarray(
        mw_kq.reshape(NJ, 128, N, GH, LH).transpose(1, 4, 3, 0, 2)).astype(f16)

    in_maps = []
    for c in range(8):
        p, s = c // 2, c % 2
        qo = 288 * s
        bsl = slice(2 * p, 2 * p + 2)
        in_maps.append({
            "xt": np.ascontiguousarray(xta[bsl].transpose(1, 0, 2, 3)),
            "xq": np.ascontiguousarray(
                xta[bsl, :, :, qo:qo + QW].transpose(1, 0, 2, 3)),
            "wq": wqp, "wk": wkp, "wv": wvp, "vm": vmp, "pw": pwp,
            "mw": np.ascontiguousarray(mw_full[:, :, :, :, qo:qo + QW]),
        })
    return in_maps


_NC3 = None


def get_nc3():
    global _NC3
    if _NC3 is None:
        _NC3 = build_nc3()
    return _NC3


def kernel(x, masks, Wq, Wk, Wv, mask_proj, proj_w, proj_b):
    x = np.asarray(x, np.float32)
    proj_b = np.asarray(proj_b, np.float32)
    in_maps = prep_inputs3(
        x, np.asarray(masks, np.float32), np.asarray(Wq, np.float32),
        np.asarray(Wk, np.float32), np.asarray(Wv, np.float32),
        np.asarray(mask_proj, np.float32), np.asarray(proj_w, np.float32),
        proj_b)
    res = bass_utils.run_bass_kernel_spmd(get_nc3(), in_maps, core_ids=list(range(8)))
    out = np.zeros((B, N, C), np.float32)
    for c in range(8):
        p, s = c // 2, c % 2
        ot = np.asarray(res.results[c]["ot"], np.float32)  # [2, 6, 128, QW]
        ot = ot.reshape(2, C, QW).transpose(0, 2, 1)       # [2, QW, C]
        for i, b in enumerate(range(2 * p, 2 * p + 2)):
            if s == 0:
                out[b, 0:289] = ot[i]
            else:
                out[b, 289:577] = ot[i, 1:]
    return (out + proj_b).astype(np.float32)


# revision 6
# speedup vs baseline: 1.1205x; 1.0069x over previous
# Trainium2 Bass kernel for nn_Attention_48052094107920 (sparse_attention).
# See build_nc3 docstring comments below for the design.
# v3: batch-pair x query-half sharding with host-folded mask weights.
#
# Core c = (batch-pair p = c//2, q-half s = c%2). Each core processes its 2
# batches for ALL 12 local heads over a 289-column query slice (s=0 covers
# q 0:289, s=1 covers q 288:577; the host drops the overlap column). Outputs
# are full projections (transposed layout); the host transposes, adds
# proj_b, and concatenates -- no cross-core reduction.
#
# Key design points vs v2:
#   - mw[g,h] = sum_m mask_proj[m, g*12+h] * masks_m is computed on the HOST
#     (batch-independent weight folding) and streamed per-head from DRAM
#     (13.3MB/core, overlapped with the head loop under the DMA-device
#     budget). Kills the on-chip DVE precompute entirely.
#   - mix at_h = sum_g S_g * mw[g,h]: products for g=1,2 on the Pool
#     (gpsimd) engine (otherwise idle, tensor ops at 1.2GHz), the g=0
#     product and both adds on DVE (fp16 2x mode).
#   - exp: ONE activation per (b,h): padded k rows have S=0 (zero-padded x)
#     and mw=0 (host zeros) -> at=0 -> e=1, and vhat rows there are 0
#     (including the ones-column that generates Z), so pads contribute
#     nothing to p@v or Z. No pad-bias pass.
#   - v projection contracts only the real 768 channels (6 steps); the
#     per-head ones-columns that generate Z during p@v come from a tiny
#     host-provided token-validity mask copied in by the Pool engine.
#   - output projection emitted transposed ([c-chunk, q]): the contraction
#     runs as 12 64-deep steps costing q-width per step, and the lhsT is the
#     proj weight directly; host undoes the transpose.
#   - head loop interleaves the two batches with batch 0 running 3 heads
#     ahead, so batch 1's phase A overlaps batch 0's first heads and every
#     in-order engine queue stays busy; mw tiles rotate through 4 buffers.
#   - softmax normalization per 4-head group: in-place reciprocal on the Z
#     row of the o/Z staging tile, DMA partition-broadcast, one multiply.

import numpy as np

import concourse.bass as bass
import concourse.bacc as bacc_mod
import concourse.mybir as mybir
import concourse.tile as tile
from concourse import bass_utils

BF = mybir.dt.float16
F32 = mybir.dt.float32
AF = mybir.ActivationFunctionType
OP = mybir.AluOpType

B, N, C = 8, 577, 768
GH, LH, ML, HD = 3, 12, 3, 64
SCALE = HD ** -0.5
NP = 640          # padded tokens (5 * 128)
NJ = 5            # k chunks of 128
KQ = 6            # contraction chunks (768 channels)
QW = 289          # query-half width (s=0: 0:289, s=1: 288:577)
VW = HD + 1       # 65: per-head v block [v | ones]


def build_nc3():
    nc = bacc_mod.Bacc("TRN2", target_bir_lowering=False, debug=False, num_devices=8)

    xt = nc.dram_tensor("xt", [128, 2, KQ, NP], BF, kind="ExternalInput")
    xq = nc.dram_tensor("xq", [128, 2, KQ, QW], BF, kind="ExternalInput")
    wq = nc.dram_tensor("wq", [128, KQ, GH * HD], BF, kind="ExternalInput")
    wk = nc.dram_tensor("wk", [128, KQ, GH * HD], BF, kind="ExternalInput")
    wv = nc.dram_tensor("wv", [128, KQ, LH * VW], BF, kind="ExternalInput")
    vm = nc.dram_tensor("vm", [128, NJ, LH], BF, kind="ExternalInput")
    pw = nc.dram_tensor("pw", [128, 6, C], BF, kind="ExternalInput")
    mw = nc.dram_tensor("mw", [128, LH, GH, NJ, QW], BF, kind="ExternalInput")
    out = nc.dram_tensor("ot", [2, 6, 128, QW], BF, kind="ExternalOutput")

    with tile.TileContext(nc) as tc, \
         tc.tile_pool(name="const", bufs=1) as cpool, \
         tc.tile_pool(name="mwst", bufs=4) as mpool, \
         tc.tile_pool(name="work", bufs=2) as wpool, \
         tc.tile_pool(name="atp", bufs=3) as atpool, \
         tc.tile_pool(name="ttp", bufs=2) as ttpool, \
         tc.tile_pool(name="ep", bufs=5) as epool, \
         tc.tile_pool(name="psA", bufs=2, space="PSUM") as ppA, \
         tc.tile_pool(name="psS", bufs=3, space="PSUM") as ppS, \
         tc.tile_pool(name="psO", bufs=2, space="PSUM") as ppO, \
         tc.tile_pool(name="psZ", bufs=1, space="PSUM") as ppZ:

        # ---- input loads, ordered so compute starts ASAP ----
        wq_s = cpool.tile([128, KQ, GH * HD], BF, tag="wq")
        nc.sync.dma_start(wq_s[:], wq.ap())
        xq_s = cpool.tile([128, 2, KQ, QW], BF, tag="xq")
        nc.sync.dma_start(xq_s[:, 0], xq.ap()[:, 0])
        wk_s = cpool.tile([128, KQ, GH * HD], BF, tag="wk")
        nc.sync.dma_start(wk_s[:], wk.ap())
        xt_s = cpool.tile([128, 2, KQ, NP], BF, tag="xt")
        nc.sync.dma_start(xt_s[:, 0], xt.ap()[:, 0])
        def mw_load(h):
            t = mpool.tile([128, GH, NJ, QW], BF, tag="mwh", name=f"mw{h}")
            nc.sync.dma_start(t[:], mw.ap()[:, h])
            return t

        mwq = {0: mw_load(0)}
        wv_s = cpool.tile([128, KQ, LH * VW], BF, tag="wv")
        nc.sync.dma_start(wv_s[:], wv.ap())
        vm_s = cpool.tile([128, NJ, LH], BF, tag="vm")
        nc.sync.dma_start(vm_s[:], vm.ap())
        mwq[1] = mw_load(1)
        nc.sync.dma_start(xq_s[:, 1], xq.ap()[:, 1])
        nc.sync.dma_start(xt_s[:, 1], xt.ap()[:, 1])
        mwq[2] = mw_load(2)
        mwq[3] = mw_load(3)

        # persistent per-batch state
        qT01 = [cpool.tile([128, QW], BF, tag=f"q01_{b}", name=f"q01_{b}") for b in range(2)]
        qT2 = [cpool.tile([64, QW], BF, tag=f"q2_{b}", name=f"q2_{b}") for b in range(2)]
        kT01 = [cpool.tile([128, NP], BF, tag=f"k01_{b}", name=f"k01_{b}") for b in range(2)]
        kT2 = [cpool.tile([64, NP], BF, tag=f"k2_{b}", name=f"k2_{b}") for b in range(2)]
        vt = [cpool.tile([128, NJ, LH * VW], BF, tag=f"vt_{b}", name=f"vt_{b}") for b in range(2)]
        ssb = [cpool.tile([128, GH, NJ, QW], BF, tag=f"ssb_{b}", name=f"ssb_{b}") for b in range(2)]
        povs = [cpool.tile([VW, LH, QW], BF, tag=f"pov_{b}", name=f"pov_{b}") for b in range(2)]
        on = [[cpool.tile([128, 2, QW], BF, tag=f"on_{b}_{g}", name=f"on_{b}_{g}")
               for g in range(3)] for b in range(2)]
        pw_box = [None]
        ones_t = cpool.tile([VW, 64], BF, tag="ones")
        nc.vector.memset(ones_t[:], 1.0)

        def qk_proj(b):
            cp = nc.scalar.copy if b == 0 else nc.vector.tensor_copy
            # q projection (289 cols), channels on partitions
            for msl, mp, dst in ((slice(0, 128), 128, qT01[b]),
                                 (slice(128, 192), 64, qT2[b])):
                ps = ppA.tile([128, 512], F32, tag="bigA", name="psA")[:mp, :QW]
                for o in range(KQ):
                    nc.tensor.matmul(ps, wq_s[:, o, msl], xq_s[:, b, o, :],
                                     start=(o == 0), stop=(o == KQ - 1))
                cp(dst[:mp, :], ps)
            # k projection (full 640; padded tokens project to 0)
            for msl, mp, dst in ((slice(0, 128), 128, kT01[b]),
                                 (slice(128, 192), 64, kT2[b])):
                for n0, n1 in ((0, 512), (512, NP)):
                    ps = ppA.tile([128, 512], F32, tag="bigA", name="psA")[:mp, : n1 - n0]
                    for o in range(KQ):
                        nc.tensor.matmul(ps, wk_s[:, o, msl], xt_s[:, b, o, n0:n1],
                                         start=(o == 0), stop=(o == KQ - 1))
                    cp(dst[:mp, n0:n1], ps)

        def v_proj(b, half):
            # v-hat projection for heads [6*half, 6*half+6): tokens on
            # partitions, interleaved [v_h | 0] blocks; the zero
            # ones-columns are then filled from vm
            cp = nc.scalar.copy
            n0, n1 = half * 6 * VW, (half + 1) * 6 * VW
            for kc in range(NJ):
                ps = ppA.tile([128, 512], F32, tag="bigA", name="psA")[:, : n1 - n0]
                for o in range(KQ):
                    nc.tensor.matmul(ps, xt_s[:, b, o, kc * 128:(kc + 1) * 128],
                                     wv_s[:, o, n0:n1],
                                     start=(o == 0), stop=(o == KQ - 1))
                cp(vt[b][:, kc, n0:n1], ps)
                nc.gpsimd.tensor_copy(vt[b][:, kc, n0 + HD:n1:VW],
                                      vm_s[:, kc, half * 6:half * 6 + 6])

        def qg(b, g):
            return (qT01[b][0:64], qT01[b][64:128], qT2[b][0:64])[g]

        def kg(b, g):
            return (kT01[b][0:64], kT01[b][64:128], kT2[b][0:64])[g]

        def scores(b):
            for g in range(GH):
                for j in range(NJ):
                    ps = ppS.tile([128, QW], F32, tag="s", name="psS")
                    nc.tensor.matmul(ps, kg(b, g)[:, j * 128:(j + 1) * 128],
                                     qg(b, g), start=True, stop=True)
                    if b == 0:
                        nc.vector.tensor_copy(ssb[b][:, g, j], ps)
                    else:
                        nc.scalar.copy(ssb[b][:, g, j], ps)

        def head(b, h, mwt):
            at = atpool.tile([128, NJ, QW], BF, tag="at")
            tb = ttpool.tile([128, NJ, QW], BF, tag="tb")
            tt = ttpool.tile([128, NJ, QW], BF, tag="tt")
            nc.gpsimd.tensor_mul(tb[:], ssb[b][:, 1], mwt[:, 1])
            nc.gpsimd.tensor_mul(tt[:], ssb[b][:, 2], mwt[:, 2])
            nc.vector.tensor_mul(at[:], ssb[b][:, 0], mwt[:, 0])
            nc.vector.tensor_add(at[:], at[:], tb[:])
            nc.vector.tensor_add(at[:], at[:], tt[:])
            e = epool.tile([128, NJ, QW], BF, tag="e")
            nc.scalar.activation(e[:], at[:], AF.Exp)
            pov = ppO.tile([VW, QW], F32, tag="ov", name="psO")
            for j in range(NJ):
                nc.tensor.matmul(pov, vt[b][:, j, h * VW:(h + 1) * VW], e[:, j, :],
                                 start=(j == 0), stop=(j == NJ - 1))
            if b == 1 and h == LH - 2:
                nc.vector.tensor_copy(povs[b][:, h], pov)
            else:
                nc.scalar.copy(povs[b][:, h], pov)

        def tail4(b, h0):
            hs = slice(h0, h0 + 4)
            g = h0 // 4
            with nc.allow_low_precision(reason="Z scaled into f16 range; 2e-2 tol"):
                nc.vector.reciprocal(povs[b][64:65, hs], povs[b][64:65, hs])
            zrep = cpool.tile([64, 4, QW], BF, tag="zrep", name="zrep")
            nc.sync.dma_start(
                zrep[:], povs[b][64:65, None, hs, :].to_broadcast((1, 64, 4, QW)))
            nc.gpsimd.tensor_mul(on[b][g][0:64], povs[b][0:64, h0:h0 + 4:2],
                                 zrep[:, 0::2])
            ot_ = wpool.tile([64, 2, QW], BF, tag="otmp")
            nc.gpsimd.tensor_mul(ot_[:], povs[b][0:64, h0 + 1:h0 + 4:2],
                                 zrep[:, 1::2])
            nc.sync.dma_start(on[b][g][64:128], ot_[:])

        def tail1(b, h):
            # per-head low-latency variant for the final exposed group:
            # PE replicates the 1/Z row into PSUM, DVE applies it
            g, p, odd = h // 4, (h % 4) // 2, h % 2
            with nc.allow_low_precision(reason="Z scaled into f16 range; 2e-2 tol"):
                nc.vector.reciprocal(povs[b][64:65, h], povs[b][64:65, h])
            zr = ppO.tile([VW, QW], F32, tag="ov", name="psO")[0:64, :]
            nc.tensor.matmul(zr, ones_t[64:65, :], povs[b][64:65, h],
                             start=True, stop=True)
            if odd:
                o1 = wpool.tile([64, QW], BF, tag="otmp1")
                nc.vector.tensor_mul(o1[:], povs[b][0:64, h], zr)
                nc.sync.dma_start(on[b][g][64:128, p], o1[:])
            else:
                nc.vector.tensor_mul(on[b][g][0:64, p], povs[b][0:64, h], zr)

        def proj(b):
            outsb = wpool.tile([128, 6, QW], BF, tag="outsb")
            pw_s = pw_box[0]
            for c0 in (0, 3):
                pss = [ppS.tile([128, QW], F32, tag="s", name=f"psP{c0+i}")
                       for i in range(3)]
                for h in range(LH):
                    for i, ps in enumerate(pss):
                        nc.tensor.matmul(
                            ps, pw_s[:, h, (c0 + i) * 128:(c0 + i + 1) * 128],
                            on[b][:, h, :],
                            start=(h == 0), stop=(h == LH - 1))
                for i, ps in enumerate(pss):
                    nc.scalar.copy(outsb[:, c0 + i], ps)
            nc.sync.dma_start(
                out.ap()[b].rearrange("c p q -> p c q"), outsb[:])

        # ---- schedule ----
        # PE warmup: tiny matmuls so the p-state ramp completes before the
        # real projection chains arrive
        for _ in range(36):
            zw = ppZ.tile([128, QW], F32, tag="zr", name="psZ")[0:64, 0:64]
            nc.tensor.matmul(zw, ones_t[64:65, :], ones_t[64:65, :],
                             start=True, stop=True)

        qk_proj(0)
        scores(0)
        v_proj(0, 0)

        def run_head(b, h):
            head(b, h, mwq[h])
            if b == 1 and h >= LH - 4:
                tail1(b, h)
            elif h % 4 == 3:
                tail4(b, h - 3)

        run_head(0, 0)
        run_head(0, 1)
        qk_proj(1)
        scores(1)
        v_proj(1, 0)
        pss0 = [None]
        outsb_box = [None]

        def close5(b, pss, outsb):
            # final 4 head-steps on the 5 open chains, then per-chunk copies
            # and immediate DMAs so the output drains as it lands
            pw_s = pw_box[0]
            for pp in range(4, 6):
                for cc, ps in enumerate(pss):
                    nc.tensor.matmul(ps, pw_s[:, pp, cc * 128:(cc + 1) * 128],
                                     on[b][pp // 2][:, pp % 2, :],
                                     start=False, stop=(pp == 5))
            for cc, ps in enumerate(pss):
                if cc % 2 == 0:
                    nc.scalar.copy(outsb[:, cc], ps)
                else:
                    nc.vector.tensor_copy(outsb[:, cc], ps)
                nc.sync.dma_start(out.ap()[b, cc], outsb[:, cc])

        def chunk5(b, outsb):
            # full-width chunk-5 chain on the psZ bank (zr lives in ppO now)
            pw_s = pw_box[0]
            ps6 = ppZ.tile([128, QW], F32, tag="zr", name="psZ")
            for pp in range(6):
                nc.tensor.matmul(ps6, pw_s[:, pp, 640:768],
                                 on[b][pp // 2][:, pp % 2, :],
                                 start=(pp == 0), stop=(pp == 5))
            nc.vector.tensor_copy(outsb[:, 5], ps6)
            nc.sync.dma_start(out.ap()[b, 5], outsb[:, 5])

        def open_steps(pss, b, p0, p1):
            # pair-steps: contraction over 128 = 2 heads x 64 dims
            pw_s = pw_box[0]
            for pp in range(p0, p1):
                for cc, ps in enumerate(pss):
                    nc.tensor.matmul(ps, pw_s[:, pp, cc * 128:(cc + 1) * 128],
                                     on[b][pp // 2][:, pp % 2, :],
                                     start=(pp == 0), stop=False)

        for h in range(2, LH):
            run_head(0, h)
            run_head(1, h - 2)
            if h == 2:
                pw_s = cpool.tile([128, 6, C], BF, tag="pw")
                nc.sync.dma_start(pw_s[:, 0:3], pw.ap()[:, 0:3])
                pw_box[0] = pw_s
            if h == 3:
                v_proj(0, 1)
                nc.sync.dma_start(pw_box[0][:, 3:6], pw.ap()[:, 3:6])
            if h == 4:
                v_proj(1, 1)
            if h + 2 < LH:
                mwq[h + 2] = mw_load(h + 2)
            if h == 8:
                pss0[0] = [ppS.tile([128, QW], F32, tag="s", name=f"psP{i}")
                           for i in range(3)]
                pss0[0] += [ppA.tile([128, 512], F32, tag="bigA",
                                     name=f"psPA{i}")[:, :QW] for i in range(2)]
                open_steps(pss0[0], 0, 0, 2)
            if h == 9:
                open_steps(pss0[0], 0, 2, 4)
            if h == 11:
                outsb_box[0] = wpool.tile([128, 6, QW], BF, tag="outsb", name="outsb0")
                chunk5(0, outsb_box[0])
        run_head(1, LH - 1)
        close5(0, pss0[0], outsb_box[0])
        pss1 = [ppS.tile([128, QW], F32, tag="s", name=f"psQ{i}")
                for i in range(3)]
        pss1 += [ppA.tile([128, 512], F32, tag="bigA", name=f"psR{i}")[:, :QW]
                 for i in range(2)]
        open_steps(pss1, 1, 0, 5)
        ps6b1 = ppZ.tile([128, QW], F32, tag="zr", name="psZ")
        for pp in range(5):
            nc.tensor.matmul(ps6b1, pw_s[:, pp, 640:768],
                             on[1][pp // 2][:, pp % 2, :],
                             start=(pp == 0), stop=False)
        run_head(1, LH - 2)
        outsb1 = wpool.tile([128, 6, QW], BF, tag="outsb")
        for cc, ps in enumerate(pss1):
            nc.tensor.matmul(ps, pw_s[:, 5, cc * 128:(cc + 1) * 128],
                             on[1][2][:, 1, :], start=False, stop=True)
        nc.tensor.matmul(ps6b1, pw_s[:, 5, 640:768], on[1][2][:, 1, :],
                         start=False, stop=True)
        for cc, ps in enumerate(pss1 + [ps6b1]):
            if cc % 2 == 0:
                nc.scalar.copy(outsb1[:, cc], ps)
            else:
                nc.vector.tensor_copy(outsb1[:, cc], ps)
        nc.sync.dma_start(
            out.ap()[1, 0:3].rearrange("c p q -> p c q"), outsb1[:, 0:3])
        nc.sync.dma_start(
            out.ap()[1, 3:6].rearrange("c p q -> p c q"), outsb1[:, 3:6])

    nc.compile()
    return nc


def prep_inputs3(x, masks, Wq, Wk, Wv, mask_proj, proj_w, proj_b):
    """Build the 8 per-core input maps."""
    f16 = np.float16

    xhatT = np.zeros((B, C, NP), np.float32)
    xhatT[:, :, :N] = x.transpose(0, 2, 1)
    xta = np.ascontiguousarray(
        xhatT.reshape(B, KQ, 128, NP).transpose(0, 2, 1, 3)).astype(f16)

    def wpad(w, scale=1.0):
        return np.ascontiguousarray(
            (w * scale).reshape(KQ, 128, -1).transpose(1, 0, 2)).astype(f16)

    wqp = wpad(Wq, SCALE)
    wkp = wpad(Wk)

    # v weights interleaved per head as [v_h (64) | zero ones-col]
    wvh = np.zeros((C, LH * VW), np.float32)
    for h in range(LH):
        wvh[:, h * VW:h * VW + HD] = Wv[:, h * HD:(h + 1) * HD]
    wvp = wpad(wvh, 1.0 / 64.0)

    # token-validity mask -> the per-head ones columns of v-hat
    vmp = np.zeros((128, NJ, LH), np.float32)
    for j in range(NJ):
        lim = min(max(N - j * 128, 0), 128)
        vmp[:lim, j, :] = 1.0 / 64.0
    vmp = vmp.astype(f16)

    pwp = np.ascontiguousarray(
        proj_w.reshape(6, 2, 64, C).transpose(1, 2, 0, 3).reshape(128, 6, C)
    ).astype(f16)

    # host-folded mask weights: [k, q, g, h] zero-padded in k
    mw_nn = (masks.reshape(-1, ML).astype(np.float64)
             @ mask_proj.astype(np.float64)).astype(np.float32)
    mw_nn = mw_nn.reshape(N, N, GH, LH)          # [q, k, g, h]
    mw_kq = np.zeros((NP, N, GH, LH), np.float32)
    mw_kq[:N] = mw_nn.transpose(1, 0, 2, 3)      # [k, q, g, h]
    mw_full = np.ascontiguousarray(
        mw_kq.reshape(NJ, 128, N, GH, LH).transpose(1, 4, 3, 0, 2)).astype(f16)

    in_maps = []
    for c in range(8):
        p, s = c // 2, c % 2
        qo = 288 * s
        bsl = slice(2 * p, 2 * p + 2)
        in_maps.append({
            "xt": np.ascontiguousarray(xta[bsl].transpose(1, 0, 2, 3)),
            "xq": np.ascontiguousarray(
                xta[bsl, :, :, qo:qo + QW].transpose(1, 0, 2, 3)),
            "wq": wqp, "wk": wkp, "wv": wvp, "vm": vmp, "pw": pwp,
            "mw": np.ascontiguousarray(mw_full[:, :, :, :, qo:qo + QW]),
        })
    return in_maps


_NC3 = None


def get_nc3():
    global _NC3
    if _NC3 is None:
        _NC3 = build_nc3()
    return _NC3


def kernel(x, masks, Wq, Wk, Wv, mask_proj, proj_w, proj_b):
    x = np.asarray(x, np.float32)
    proj_b = np.asarray(proj_b, np.float32)
    in_maps = prep_inputs3(
        x, np.asarray(masks, np.float32), np.asarray(Wq, np.float32),
        np.asarray(Wk, np.float32), np.asarray(Wv, np.float32),
        np.asarray(mask_proj, np.float32), np.asarray(proj_w, np.float32),
        proj_b)
    res = bass_utils.run_bass_kernel_spmd(get_nc3(), in_maps, core_ids=list(range(8)))
    out = np.zeros((B, N, C), np.float32)
    for c in range(8):
        p, s = c // 2, c % 2
        ot = np.asarray(res.results[c]["ot"], np.float32)  # [2, 6, 128, QW]
        ot = ot.reshape(2, C, QW).transpose(0, 2, 1)       # [2, QW, C]
        for i, b in enumerate(range(2 * p, 2 * p + 2)):
            if s == 0:
                out[b, 0:289] = ot[i]
            else:
                out[b, 289:577] = ot[i, 1:]
    return (out + proj_b).astype(np.float32)
